# revision 11
# baseline (speedup 1.0000x reference)
"""CPGA Trainium2 Bass kernel.

Two SPMD launches over 8 NeuronCores, sharded (batch, row-half):
  stage 1: LN(low)/LN(high) -> fused -> mask logits + aligned features ->
           per-core partial class-prototype sums (streaming spatial softmax).
  host   : combine partials -> cf -> memory mix -> k/v + folded biases.
  stage 2: query conv path, cross-attention over 19 prototypes, proj+residual,
           LN, conv-FFN (1x1 -> depthwise 3x3 -> gelu -> 1x1), final residual.

Layout: channels on partitions, pixels on the free dim (all SBUF tiles are
partition-first). Depthwise 3x3 convs run on TensorE as 9 diagonal matmuls
with shifted rhs slices over halo-padded tiles. LayerNorm stats use
ones-matmul partition broadcasts; rstd = exp(-0.5*ln(var+eps)).
"""

import numpy as np
import ml_dtypes

import concourse.bass as bass
import concourse.mybir as mybir
from concourse import bacc
from concourse.tile import TileContext
from concourse.bass_utils import run_bass_kernel_spmd

# Prefer the combined Ln+Exp activation table so per-tile Ln/Exp/Square
# sequences don't thrash ACT_TABLE_LOADs (the insertion pass picks the
# first table containing each func).
from concourse import hw_specs as _hw_specs
_orig_get_act_tables = _hw_specs.get_activation_tables

def _act_tables_combined_first(arch):
    # Keep canonical order/indices (walrus maps set-id by index); advertise
    # only the two tables we want selected so the greedy pass never picks a
    # set lacking a func we'll need next (Ln+Exp+Square live together in
    # natural_log_exp_and_others; Gelu needs gelu_and_others).
    tabs = dict(_orig_get_act_tables(arch))
    keep = ("natural_log_exp_and_others", "gelu_and_others")
    return {k: (v if k in keep else type(v)()) for k, v in tabs.items()}

bacc.get_activation_tables = _act_tables_combined_first

BF = mybir.dt.bfloat16
F32 = mybir.dt.float32
F32R = mybir.dt.float32r
AL = mybir.AluOpType
AF = mybir.ActivationFunctionType
bf16 = ml_dtypes.bfloat16

B, C, H, W = 4, 256, 128, 128
NCL, NH, HD = 19, 8, 32
SCALE = HD ** -0.5
MOM = 0.1
EPS = 1e-5
NCORES = 8
R = 64            # rows per core chunk
S1_T = 16         # stage-1 tiles of 512 px (64 rows)
S2_T = 17         # stage-2 tiles (68 rows incl 2-row halo each side)
TN = 512          # pixels per tile
QH = 130          # free-dim halo for dw conv tiles (even -> 4x DVE copies)
LN2 = float(np.log(0.5))

GP_APPLY = True   # put the LN-apply multiplies on GPSIMD
TAP_ORDER = [(dr, 0) for dr in (-1, 0, 1)] + [(dr, dc) for dc in (-1, 1) for dr in (-1, 0, 1)]


# ----------------------------------------------------------------------------
# stage 1 builder
# ----------------------------------------------------------------------------

def build_stage1():
    nc = bacc.Bacc()
    lo = nc.dram_tensor("lo", [128, 2, S1_T * TN], BF, kind="ExternalInput")
    hi = nc.dram_tensor("hi", [128, 2, S1_T * TN], BF, kind="ExternalInput")
    ones = nc.dram_tensor("ones", [128, 128], BF, kind="ExternalInput")
    ident = nc.dram_tensor("ident", [128, 128], BF, kind="ExternalInput")
    wm1 = nc.dram_tensor("wm1", [128, 2, 128], BF, kind="ExternalInput")
    wm2 = nc.dram_tensor("wm2", [128, 2, NCL], BF, kind="ExternalInput")
    wal = nc.dram_tensor("wal", [128, 2, 128], BF, kind="ExternalInput")
    S_out = nc.dram_tensor("S_out", [NCL, 256], F32, kind="ExternalOutput")
    Z_out = nc.dram_tensor("Z_out", [NCL, 1], F32, kind="ExternalOutput")

    with TileContext(nc) as tc:
        with (
            tc.tile_pool(name="cst", bufs=1) as cst,
            tc.tile_pool(name="sb", bufs=3) as sb,
            tc.tile_pool(name="st", bufs=2) as st,
            tc.tile_pool(name="ps_a", bufs=1, space="PSUM") as ps_a,
            tc.tile_pool(name="ps_b", bufs=2, space="PSUM") as ps_b,
        ):
            ps_st = ps_a; ps_cv = ps_b; ps_tp = ps_b; ps_cf = ps_a
            ones_t = cst.tile([128, 128], BF, tag="ones")
            nc.sync.dma_start(ones_t[:], ones[:])
            ident_t = cst.tile([128, 128], BF, tag="ident")
            nc.sync.dma_start(ident_t[:], ident[:])
            wm1_t = cst.tile([128, 2, 128], BF, tag="wm1")
            nc.sync.dma_start(wm1_t[:], wm1[:])
            wm2_t = cst.tile([128, 2, NCL], BF, tag="wm2")
            nc.sync.dma_start(wm2_t[:], wm2[:])
            wal_t = cst.tile([128, 2, 128], BF, tag="wal")
            nc.sync.dma_start(wal_t[:], wal[:])
            zbuf = cst.tile([NCL, S1_T], F32, tag="zbuf")
            cf = ps_cf.tile([NCL, 256], F32, tag="cf")
            epsb = cst.tile([128, 1], F32, tag="epsb")
            nc.vector.memset(epsb[:], EPS)
            ln2b = cst.tile([128, 1], F32, tag="ln2b")
            nc.vector.memset(ln2b[:], LN2)

            for t in range(S1_T):
                sl = slice(t * TN, (t + 1) * TN)
                lo_t = sb.tile([128, 2, TN], BF, tag="lo")
                nc.sync.dma_start(lo_t[:], lo[:, :, sl])
                hi_t = sb.tile([128, 2, TN], BF, tag="hi")
                nc.sync.dma_start(hi_t[:], hi[:, :, sl])

                sql = sb.tile([128, 2, TN], BF, tag="sql")
                sqh = sb.tile([128, 2, TN], BF, tag="sqh")
                for ct in range(2):
                    nc.vector.tensor_tensor(sql[:, ct, :], lo_t[:, ct, :], lo_t[:, ct, :], op=AL.mult)
                    nc.vector.tensor_tensor(sqh[:, ct, :], hi_t[:, ct, :], hi_t[:, ct, :], op=AL.mult)

                def ln_stats(x_t, sq_t, tag):
                    s1 = ps_st.tile([128, TN], F32, tag="s1")
                    nc.tensor.matmul(s1[:], ones_t[:], x_t[:, 0, :], start=True, stop=False)
                    nc.tensor.matmul(s1[:], ones_t[:], x_t[:, 1, :], start=False, stop=True)
                    s2 = ps_st.tile([128, TN], F32, tag="s2")
                    nc.tensor.matmul(s2[:], ones_t[:], sq_t[:, 0, :], start=True, stop=False)
                    nc.tensor.matmul(s2[:], ones_t[:], sq_t[:, 1, :], start=False, stop=True)
                    mu2 = st.tile([128, TN], F32, tag="mu2" + tag)
                    nc.scalar.activation(mu2[:], s1[:], AF.Square, scale=1.0 / C)
                    var = st.tile([128, TN], F32, tag="var" + tag)
                    nc.vector.scalar_tensor_tensor(var[:], s2[:], 1.0 / C, mu2[:],
                                                   op0=AL.mult, op1=AL.subtract)
                    lnv = st.tile([128, TN], F32, tag="ln" + tag)
                    nc.scalar.activation(lnv[:], var[:], AF.Ln, bias=epsb[:])
                    r2 = st.tile([128, TN], BF, tag="r2" + tag)   # 0.5 * rstd
                    nc.scalar.activation(r2[:], lnv[:], AF.Exp, scale=-0.5, bias=ln2b[:])
                    m2 = st.tile([128, TN], BF, tag="m2" + tag)   # 0.5 * mu * rstd
                    nc.vector.scalar_tensor_tensor(m2[:], s1[:], 1.0 / C, r2[:],
                                                   op0=AL.mult, op1=AL.mult)
                    return r2, m2

                rl2, m2l = ln_stats(lo_t, sql, "l")
                rh2, m2h = ln_stats(hi_t, sqh, "h")
                m12 = st.tile([128, TN], BF, tag="m12")
                nc.vector.tensor_tensor(m12[:], m2l[:], m2h[:], op=AL.add)

                # fused = low*rl2 + high*rh2 - m12
                f_t = sb.tile([128, 2, TN], BF, tag="f")
                t1 = sb.tile([128, 2, TN], BF, tag="t1")
                t2 = sb.tile([128, 2, TN], BF, tag="t2")
                eng = nc.gpsimd if GP_APPLY else nc.vector
                for ct in range(2):
                    eng.tensor_tensor(t1[:, ct, :], lo_t[:, ct, :], rl2[:], op=AL.mult)
                    eng.tensor_tensor(t2[:, ct, :], hi_t[:, ct, :], rh2[:], op=AL.mult)
                for ct in range(2):
                    nc.vector.tensor_tensor(f_t[:, ct, :], t1[:, ct, :], t2[:, ct, :], op=AL.add)
                    nc.vector.tensor_tensor(f_t[:, ct, :], f_t[:, ct, :], m12[:], op=AL.subtract)

                # mask conv1 (block-diag grouped) + align conv
                c1a = sb.tile([128, 2, TN], BF, tag="c1a")
                xa = sb.tile([128, 2, TN], BF, tag="xa")
                for ct in range(2):
                    c1p = ps_cv.tile([128, TN], F32, tag="cv")
                    nc.tensor.matmul(c1p[:], wm1_t[:, ct, :], f_t[:, ct, :], start=True, stop=True)
                    nc.scalar.copy(c1a[:, ct, :], c1p[:])
                    alp = ps_cv.tile([128, TN], F32, tag="cv")
                    nc.tensor.matmul(alp[:], wal_t[:, ct, :], f_t[:, ct, :], start=True, stop=True)
                    if ct == 0:
                        nc.scalar.copy(xa[:, ct, :], alp[:])
                    else:
                        nc.vector.tensor_copy(xa[:, ct, :], alp[:])

                # mask logits -> exp (no max-sub; logits are tiny by construction)
                mk = ps_a.tile([NCL, TN], F32, tag="mk")
                nc.tensor.matmul(mk[:], wm2_t[:, 0, :], c1a[:, 0, :], start=True, stop=False)
                nc.tensor.matmul(mk[:], wm2_t[:, 1, :], c1a[:, 1, :], start=False, stop=True)
                e_t = sb.tile([NCL, TN], BF, tag="e")
                nc.scalar.activation(e_t[:], mk[:], AF.Exp, accum_out=zbuf[:, t:t + 1])

                # cf += e @ xa^T via per-128px-block transposes
                for blk in range(4):
                    bsl = slice(blk * 128, (blk + 1) * 128)
                    eTp = ps_tp.tile([128, 128], BF, tag="tp", name="eTp")[:, 0:NCL]
                    nc.tensor.transpose(eTp[:], e_t[:, bsl], ident_t[0:NCL, 0:NCL])
                    eTs = sb.tile([128, NCL], BF, tag="eTs")
                    nc.vector.tensor_copy(eTs[:], eTp[:])
                    xaTs = sb.tile([128, 256], BF, tag="xaTs")
                    for ct in range(2):
                        xTp = ps_tp.tile([128, 128], BF, tag="tp")
                        nc.tensor.transpose(xTp[:], xa[:, ct, bsl], ident_t[:])
                        if ct == 0:
                            nc.scalar.copy(xaTs[:, 0:128], xTp[:])
                        else:
                            nc.vector.tensor_copy(xaTs[:, 128:256], xTp[:])
                    nc.tensor.matmul(
                        cf[:], eTs[:], xaTs[:],
                        start=(t == 0 and blk == 0),
                        stop=(t == S1_T - 1 and blk == 3),
                    )

            S_sb = cst.tile([NCL, 256], F32, tag="S_sb")
            nc.vector.tensor_copy(S_sb[:], cf[:])
            nc.sync.dma_start(S_out[:], S_sb[:])
            z_sb = cst.tile([NCL, 1], F32, tag="z_sb")
            nc.vector.tensor_reduce(z_sb[:], zbuf[:], axis=mybir.AxisListType.X, op=AL.add)
            nc.sync.dma_start(Z_out[:], z_sb[:])

    nc.finalize()
    return nc


# ----------------------------------------------------------------------------
# stage 2 builder
# ----------------------------------------------------------------------------

def build_stage2():
    """Fully fused stage 2: one software-pipelined loop per tile.

    iter t: build_query(t) | attn(t-1) | build_z(t-2) | ffn(t-3).
    Per-tile LN stats (ones-matmul broadcast + Ln/Exp from the combined act
    table); out/yl kept as rotating bf16 SBUF tiles; FFN matmuls interleave
    with the attention chain so TensorE never idles past the HAM window.
    """
    nc = bacc.Bacc()
    NPX = S2_T * TN
    lo16 = nc.dram_tensor("lo16", [128, 2, NPX], BF, kind="ExternalInput")
    ones = nc.dram_tensor("ones", [128, 128], BF, kind="ExternalInput")
    kbd = nc.dram_tensor("kbd", [128, 2, 152], BF, kind="ExternalInput")
    vbd = nc.dram_tensor("vbd", [128, 256], BF, kind="ExternalInput")
    onesbd = nc.dram_tensor("onesbd", [128, 4], BF, kind="ExternalInput")
    expd = nc.dram_tensor("expd", [4, 76], F32, kind="ExternalInput")
    bexp = nc.dram_tensor("bexp", [128, 2], F32, kind="ExternalInput")
    wqdw = nc.dram_tensor("wqdw", [128, 2, 9, 128], BF, kind="ExternalInput")
    wqpw = nc.dram_tensor("wqpw", [128, 2, 256], BF, kind="ExternalInput")
    wproj = nc.dram_tensor("wproj", [128, 2, 256], BF, kind="ExternalInput")
    wmlp1 = nc.dram_tensor("wmlp1", [128, 2, 1024], BF, kind="ExternalInput")
    wdwm = nc.dram_tensor("wdwm", [128, 8, 9, 128], BF, kind="ExternalInput")
    wmlp2 = nc.dram_tensor("wmlp2", [128, 8, 256], BF, kind="ExternalInput")
    bprj = nc.dram_tensor("bprj", [128, 2], F32, kind="ExternalInput")
    b1 = nc.dram_tensor("b1", [128, 8], F32, kind="ExternalInput")
    bdw = nc.dram_tensor("bdw", [128, 8], F32, kind="ExternalInput")
    b2 = nc.dram_tensor("b2", [128, 2], F32, kind="ExternalInput")
    zmask = nc.dram_tensor("zmask", [128, 2, TN], BF, kind="ExternalInput")
    OUT = nc.dram_tensor("OUT", [128, 2, S1_T * TN], F32, kind="ExternalOutput")

    with TileContext(nc) as tc:
        with (
            tc.tile_pool(name="cst", bufs=1) as cst,
            tc.tile_pool(name="lop", bufs=3) as lop,
            tc.tile_pool(name="sb2", bufs=2) as sb2,
            tc.tile_pool(name="qp", bufs=3) as qp,
            tc.tile_pool(name="zp", bufs=3) as zp,
            tc.tile_pool(name="outp", bufs=4) as outp,
            tc.tile_pool(name="ylp", bufs=3) as ylp,
            tc.tile_pool(name="ps", bufs=2, space="PSUM") as ps,
        ):
            ones_t = cst.tile([128, 128], BF, tag="ones"); nc.sync.dma_start(ones_t[:], ones[:])
            kbd_t = cst.tile([128, 2, 152], BF, tag="kbd"); nc.sync.dma_start(kbd_t[:], kbd[:])
            vbd_t = cst.tile([128, 256], BF, tag="vbd"); nc.sync.dma_start(vbd_t[:], vbd[:])
            obd_t = cst.tile([128, 4], BF, tag="obd"); nc.sync.dma_start(obd_t[:], onesbd[:])
            expd_t = cst.tile([4, 76], F32, tag="expd"); nc.sync.dma_start(expd_t[:], expd[:])
            bexp_t = cst.tile([128, 2], F32, tag="bexp"); nc.sync.dma_start(bexp_t[:], bexp[:])
            wqdw_t = cst.tile([128, 2, 9, 128], BF, tag="wqdw"); nc.sync.dma_start(wqdw_t[:], wqdw[:])
            wqpw_t = cst.tile([128, 2, 256], BF, tag="wqpw"); nc.sync.dma_start(wqpw_t[:], wqpw[:])
            wproj_t = cst.tile([128, 2, 256], BF, tag="wproj"); nc.sync.dma_start(wproj_t[:], wproj[:])
            wmlp1_t = cst.tile([128, 2, 1024], BF, tag="wmlp1"); nc.sync.dma_start(wmlp1_t[:], wmlp1[:])
            wdwm_t = cst.tile([128, 8, 9, 128], BF, tag="wdwm"); nc.sync.dma_start(wdwm_t[:], wdwm[:])
            wmlp2_t = cst.tile([128, 8, 256], BF, tag="wmlp2"); nc.sync.dma_start(wmlp2_t[:], wmlp2[:])
            bprj_t = cst.tile([128, 2], F32, tag="bprj"); nc.sync.dma_start(bprj_t[:], bprj[:])
            b1_t = cst.tile([128, 8], F32, tag="b1"); nc.sync.dma_start(b1_t[:], b1[:])
            bdw_t = cst.tile([128, 8], F32, tag="bdw"); nc.sync.dma_start(bdw_t[:], bdw[:])
            b2_t = cst.tile([128, 2], F32, tag="b2"); nc.sync.dma_start(b2_t[:], b2[:])
            zm_t = cst.tile([128, 2, TN], BF, tag="zm"); nc.sync.dma_start(zm_t[:], zmask[:])
            epsb = cst.tile([128, 1], F32, tag="epsb")
            nc.vector.memset(epsb[:], EPS)

            qtiles = {}
            ztiles = {}
            lo_a = {}
            out_a = {}
            yl_a = {}
            e_a = {}
            rz_a = {}
            qsb_a = {}
            f01_a = {}

            # per-tile LN stats -> (rstd, mu*rstd) bf16 full-width tiles
            def ln_tile(x0, x1, nm):
                sq = sb2.tile([128, 2, TN], BF, tag="sq", name="sq" + nm)
                nc.gpsimd.tensor_tensor(sq[:, 0, :], x0, x0, op=AL.mult)
                nc.gpsimd.tensor_tensor(sq[:, 1, :], x1, x1, op=AL.mult)
                s1 = ps.tile([128, TN], F32, tag="st", name="s1" + nm)
                nc.tensor.matmul(s1[:], ones_t[:], x0, start=True, stop=False)
                nc.tensor.matmul(s1[:], ones_t[:], x1, start=False, stop=True)
                s2 = ps.tile([128, TN], F32, tag="st", name="s2" + nm)
                nc.tensor.matmul(s2[:], ones_t[:], sq[:, 0, :], start=True, stop=False)
                nc.tensor.matmul(s2[:], ones_t[:], sq[:, 1, :], start=False, stop=True)
                mu2 = sb2.tile([128, TN], BF, tag="mu2", name="mu2" + nm)
                nc.scalar.activation(mu2[:], s1[:], AF.Square, scale=1.0 / C)
                mu_b = sb2.tile([128, TN], BF, tag="mu_b", name="mu_b" + nm)
                nc.scalar.activation(mu_b[:], s1[:], AF.Identity, scale=1.0 / C)
                var = sb2.tile([128, TN], F32, tag="var", name="var" + nm)
                nc.vector.scalar_tensor_tensor(var[:], s2[:], 1.0 / C, mu2[:],
                                               op0=AL.mult, op1=AL.subtract)
                nc.scalar.activation(var[:], var[:], AF.Ln, bias=epsb[:])
                rl = sb2.tile([128, TN], BF, tag="rl", name="rl" + nm)
                nc.scalar.activation(rl[:], var[:], AF.Exp, scale=-0.5)
                m2 = sb2.tile([128, TN], BF, tag="m2", name="m2" + nm)
                nc.vector.tensor_tensor(m2[:], mu_b[:], rl[:], op=AL.mult)
                return rl, m2

            def build_query(t):
                sl = slice(t * TN, (t + 1) * TN)
                lo_t = lop.tile([128, 2, TN], BF, tag="lo", name="lo_t")
                nc.sync.dma_start(lo_t[:], lo16[:, :, sl])
                lo_a[t] = lo_t
                rl, m2 = ln_tile(lo_t[:, 0, :], lo_t[:, 1, :], "q")
                qt = qp.tile([128, 2, 2 * QH + TN], BF, tag="qt")
                qtiles[t] = qt
                for ct in range(2):
                    nc.gpsimd.tensor_tensor(qt[:, ct, QH:QH + TN], lo_t[:, ct, :], rl[:],
                                            op=AL.mult)
                for ct in range(2):
                    nc.vector.tensor_tensor(qt[:, ct, QH:QH + TN], qt[:, ct, QH:QH + TN],
                                            m2[:], op=AL.subtract)
                if t == 0:
                    nc.vector.memset(qt[:, :, 0:QH], 0.0)
                else:
                    nc.vector.tensor_copy(qt[:, :, 0:QH], qtiles[t - 1][:, :, TN:TN + QH])
                    nc.vector.tensor_copy(qtiles[t - 1][:, :, QH + TN:], qt[:, :, QH:2 * QH])
                if t == S2_T - 1:
                    nc.vector.memset(qt[:, :, QH + TN:], 0.0)

            def dw9(psum, wtile, src):
                for ti, (dr, dc) in enumerate(TAP_ORDER):
                    tap = (dr + 1) * 3 + (dc + 1)
                    off0 = QH + dr * 128
                    lhs = wtile[:, tap, :]
                    if dc == 0:
                        nc.tensor.matmul(psum[:], lhs, src[:, off0:off0 + TN],
                                         start=(ti == 0), stop=(ti == 8))
                    else:
                        rhs3 = src[:, off0:off0 + TN].rearrange("p (r w) -> p r w", w=128)
                        out3 = psum[:].rearrange("p (r w) -> p r w", w=128)
                        if dc == -1:
                            nc.tensor.matmul(out3[:, :, 1:128], lhs, rhs3[:, :, 0:127],
                                             start=False, stop=(ti == 8))
                        else:
                            nc.tensor.matmul(out3[:, :, 0:127], lhs, rhs3[:, :, 1:128],
                                             start=False, stop=(ti == 8))

            def attn_qk(s):
                qt = qtiles[s]
                qd = sb2.tile([128, 2, TN], BF, tag="qd")
                for ct in range(2):
                    qdp = ps.tile([128, TN], F32, tag="mm", name="qdp")
                    dw9(qdp, wqdw_t[:, ct], qt[:, ct, :])
                    if ct == 0:
                        nc.scalar.copy(qd[:, ct, :], qdp[:])
                    else:
                        nc.vector.tensor_copy(qd[:, ct, :], qdp[:])
                q_sb = sb2.tile([128, 2, TN], BF, tag="q_sb")
                qsb_a[s] = q_sb
                for mt in range(2):
                    qpp = ps.tile([128, TN], F32, tag="mm", name="qpp")
                    for kt in range(2):
                        nc.tensor.matmul(qpp[:], wqpw_t[:, kt, mt * 128:(mt + 1) * 128],
                                         qd[:, kt, :], start=(kt == 0), stop=(kt == 1))
                    if mt == 0:
                        nc.scalar.copy(q_sb[:, mt, :], qpp[:])
                    else:
                        nc.vector.tensor_copy(q_sb[:, mt, :], qpp[:])
                e_ab = []
                for hf in range(2):
                    lp = ps.tile([128, TN], F32, tag="at", name="lp", bufs=1)
                    for kt in range(2):
                        nc.tensor.matmul(lp[0:76, :], kbd_t[:, kt, hf * 76:hf * 76 + 76],
                                         q_sb[:, kt, :], start=(kt == 0), stop=(kt == 1))
                    e_h = sb2.tile([76, TN], BF, tag="eh%d" % hf)
                    nc.scalar.activation(e_h[:], lp[0:76, :], AF.Exp, scale=-SCALE,
                                         bias=bexp_t[0:76, hf:hf + 1])
                    e_ab.append(e_h)
                e_a[s] = e_ab
                rz = sb2.tile([4, 2, TN], F32, tag="rz", bufs=2)
                rz_a[s] = rz
                for hf in range(2):
                    zp_ = ps.tile([4, TN], F32, tag="zps", name="zp_", bufs=1)
                    nc.tensor.matmul(zp_[:], obd_t[0:76, :], e_ab[hf][:], start=True, stop=True)
                    nc.vector.reciprocal_approx_fast(rz[:, hf, :], zp_[:])

            def attn_av(s):
                e_ab = e_a.pop(s)
                rz = rz_a.pop(s)
                del qsb_a[s]
                av = sb2.tile([128, 2, TN], BF, tag="av")
                for hf in range(2):
                    rzx = ps.tile([128, TN], F32, tag="at", name="rzx", bufs=1)
                    nc.tensor.matmul(rzx[0:76, :], expd_t[:], rz[:, hf, :],
                                     start=True, stop=True)
                    en = sb2.tile([76, TN], BF, tag="en%d" % hf)
                    nc.vector.tensor_tensor(en[:], e_ab[hf][:], rzx[0:76, :], op=AL.mult)
                    avp = ps.tile([128, TN], F32, tag="mm", name="avp")
                    nc.tensor.matmul(avp[:], vbd_t[0:76, hf * 128:(hf + 1) * 128], en[:],
                                     start=True, stop=True)
                    if hf == 0:
                        nc.scalar.copy(av[:, hf, :], avp[:])
                    else:
                        nc.vector.tensor_copy(av[:, hf, :], avp[:])
                out_t = outp.tile([128, 2, TN], BF, tag="out")
                out_a[s] = out_t
                for mt in range(2):
                    op_ = ps.tile([128, TN], F32, tag="mm", name="op_")
                    for kt in range(2):
                        nc.tensor.matmul(op_[:], wproj_t[:, kt, mt * 128:(mt + 1) * 128],
                                         av[:, kt, :], start=(kt == 0), stop=(kt == 1))
                    nc.vector.scalar_tensor_tensor(out_t[:, mt, :], op_[:],
                                                   bprj_t[:, mt:mt + 1],
                                                   lo_a[s][:, mt, :], op0=AL.add, op1=AL.add)
                del lo_a[s]

            def attn_ln(s):
                out_t = out_a[s]
                ro, m2o = ln_tile(out_t[:, 0, :], out_t[:, 1, :], "o")
                yl_t = ylp.tile([128, 2, TN], BF, tag="yl")
                yl_a[s] = yl_t
                for ct in range(2):
                    nc.gpsimd.tensor_tensor(yl_t[:, ct, :], out_t[:, ct, :], ro[:],
                                            op=AL.mult)
                for ct in range(2):
                    nc.vector.tensor_tensor(yl_t[:, ct, :], yl_t[:, ct, :], m2o[:],
                                            op=AL.subtract)

            def build_z(t):
                yl_t = yl_a[t]
                zt = zp.tile([128, 8, 2 * QH + TN], BF, tag="zt")
                ztiles[t] = zt
                for g in range(8):
                    m1p = ps.tile([128, TN], F32, tag="mm", name="m1p")
                    for kt in range(2):
                        nc.tensor.matmul(m1p[:], wmlp1_t[:, kt, g * 128:(g + 1) * 128],
                                         yl_t[:, kt, :], start=(kt == 0), stop=(kt == 1))
                    if g % 2 == 0:
                        nc.scalar.activation(zt[:, g, QH:QH + TN], m1p[:], AF.Identity,
                                             bias=b1_t[:, g:g + 1])
                    else:
                        nc.vector.tensor_scalar(zt[:, g, QH:QH + TN], m1p[:],
                                                b1_t[:, g:g + 1], None, op0=AL.add)
                    if t == 0:
                        nc.vector.tensor_tensor(zt[:, g, QH:QH + TN], zt[:, g, QH:QH + TN],
                                                zm_t[:, 0, :], op=AL.mult)
                    elif t == S2_T - 1:
                        nc.vector.tensor_tensor(zt[:, g, QH:QH + TN], zt[:, g, QH:QH + TN],
                                                zm_t[:, 1, :], op=AL.mult)
                del yl_a[t]
                if t == 0:
                    nc.vector.memset(zt[:, :, 0:QH], 0.0)
                else:
                    nc.vector.tensor_copy(zt[:, :, 0:QH], ztiles[t - 1][:, :, TN:TN + QH])
                    nc.vector.tensor_copy(ztiles[t - 1][:, :, QH + TN:], zt[:, :, QH:2 * QH])
                if t == S2_T - 1:
                    nc.vector.memset(zt[:, :, QH + TN:], 0.0)

            def ffn_groups(s, g_lo, g_hi):
                zt = ztiles[s]
                if g_lo == 0:
                    f0 = ps.tile([128, TN], F32, tag="f01", name="f0")
                    f1 = ps.tile([128, TN], F32, tag="f01", name="f1")
                    f01_a[s] = (f0, f1)
                f0, f1 = f01_a[s]
                for g in range(g_lo, g_hi):
                    dwp = ps.tile([128, TN], F32, tag="mm", name="dwp")
                    dw9(dwp, wdwm_t[:, g], zt[:, g, :])
                    gel = sb2.tile([128, TN], BF, tag="gel")
                    nc.scalar.activation(gel[:], dwp[:], AF.Gelu, bias=bdw_t[:, g:g + 1])
                    nc.tensor.matmul(f0[:], wmlp2_t[:, g, 0:128], gel[:],
                                     start=(g == 0), stop=(g == 7))
                    nc.tensor.matmul(f1[:], wmlp2_t[:, g, 128:256], gel[:],
                                     start=(g == 0), stop=(g == 7))

            def ffn_fin(s):
                f0, f1 = f01_a.pop(s)
                if s == 0:
                    px0, px1, o0 = 256, TN, 0
                elif s == S2_T - 1:
                    px0, px1, o0 = 0, 256, (S2_T - 1) * TN - 256
                else:
                    px0, px1, o0 = 0, TN, s * TN - 256
                n = px1 - px0
                for ct, fps in enumerate((f0, f1)):
                    fin = sb2.tile([128, TN], F32, tag="fin", name="fin")
                    nc.vector.scalar_tensor_tensor(fin[:, 0:n], fps[:, px0:px1],
                                                   b2_t[:, ct:ct + 1],
                                                   out_a[s][:, ct, px0:px1],
                                                   op0=AL.add, op1=AL.add)
                    nc.sync.dma_start(OUT[:, ct, o0:o0 + n], fin[:, 0:n])
                del out_a[s]

            # software pipeline: query(t) | z(t-3) | attn(t-2) | ffn(t-4),
            # emission interleaved so the in-order PE queue always has dw-conv
            # work to chew on while the attention/LN chains run on ACT/DVE.
            for t in range(S2_T + 4):
                if t < S2_T:
                    build_query(t)
                if 3 <= t < S2_T + 3:
                    build_z(t - 3)
                if 2 <= t < S2_T + 2:
                    attn_qk(t - 2)
                if 4 <= t < S2_T + 4:
                    ffn_groups(t - 4, 0, 4)
                if 2 <= t < S2_T + 2:
                    attn_av(t - 2)
                if 4 <= t < S2_T + 4:
                    ffn_groups(t - 4, 4, 8)
                if 2 <= t < S2_T + 2:
                    attn_ln(t - 2)
                if 4 <= t < S2_T + 4:
                    ffn_fin(t - 4)
                    del ztiles[t - 4]
                if 2 <= t < S2_T + 2:
                    del qtiles[t - 2]

    nc.finalize()
    return nc


# ----------------------------------------------------------------------------
# host packing
# ----------------------------------------------------------------------------

def _chunk(x, b, r0, r1, pad_lo, pad_hi):
    """x[b] rows [r0-pad_lo, r1+pad_hi) zero-clamped -> [128, 2, n*128]."""
    lo_pad = np.zeros((C, pad_lo, W), np.float32)
    hi_pad = np.zeros((C, pad_hi, W), np.float32)
    lo_src = x[b, :, max(r0 - pad_lo, 0):r0, :]
    if lo_src.shape[1] > 0:
        lo_pad[:, pad_lo - lo_src.shape[1]:, :] = lo_src
    hi_src = x[b, :, r1:min(r1 + pad_hi, H), :]
    if hi_src.shape[1] > 0:
        hi_pad[:, :hi_src.shape[1], :] = hi_src
    full = np.concatenate([lo_pad, np.asarray(x[b, :, r0:r1, :], np.float32), hi_pad], axis=1)
    n = full.shape[1]
    return np.ascontiguousarray(full.reshape(2, 128, n * W).transpose(1, 0, 2))


def _bcast_rowsel():
    m = np.zeros((128, 4 * 128), np.float32)
    for i, r in enumerate((0, 32, 64, 96)):
        m[r, i * 128:(i + 1) * 128] = 1.0
    return m.astype(bf16)


_S1 = None
_S2 = None
_last_s1_inputs = None
_last_s2_inputs = None


def kernel(**inp):
    global _S1, _S2
    f32 = np.float32
    low = np.asarray(inp["low"], f32)
    high = np.asarray(inp["high"], f32)
    g_low = np.asarray(inp["g_low"], f32); b_low = np.asarray(inp["b_low"], f32)
    g_high = np.asarray(inp["g_high"], f32); b_high = np.asarray(inp["b_high"], f32)
    g_mlp = np.asarray(inp["g_mlp"], f32); b_mlp = np.asarray(inp["b_mlp"], f32)
    w_q_dw = np.asarray(inp["w_q_dw"], f32); b_q_dw = np.asarray(inp["b_q_dw"], f32)
    w_q_pw = np.asarray(inp["w_q_pw"], f32)[:, :, 0, 0]; b_q_pw = np.asarray(inp["b_q_pw"], f32)
    w_ml1 = np.asarray(inp["w_ml1"], f32)[:, :, 0, 0]
    w_ml2 = np.asarray(inp["w_ml2"], f32)[:, :, 0, 0]
    w_align = np.asarray(inp["w_align"], f32)[:, :, 0, 0]
    w_kv = np.asarray(inp["w_kv"], f32); b_kv = np.asarray(inp["b_kv"], f32)
    memory = np.asarray(inp["memory"], f32)
    w_proj = np.asarray(inp["w_proj"], f32)[:, :, 0, 0]; b_proj = np.asarray(inp["b_proj"], f32)
    w_mlp1 = np.asarray(inp["w_mlp1"], f32)[:, :, 0, 0]; b_mlp1 = np.asarray(inp["b_mlp1"], f32)
    w_mlp_dw = np.asarray(inp["w_mlp_dw"], f32); b_mlp_dw = np.asarray(inp["b_mlp_dw"], f32)
    w_mlp2 = np.asarray(inp["w_mlp2"], f32)[:, :, 0, 0]; b_mlp2 = np.asarray(inp["b_mlp2"], f32)

    assert np.allclose(g_low, g_high), "kernel requires g_low == g_high"

    def dense_grouped(wg, groups):
        o, ipg = wg.shape
        d = np.zeros((o, ipg * groups), f32)
        opg = o // groups
        for g in range(groups):
            d[g * opg:(g + 1) * opg, g * ipg:(g + 1) * ipg] = wg[g * opg:(g + 1) * opg]
        return d

    Wm1 = dense_grouped(w_ml1, 4)
    Wal = dense_grouped(w_align, 4)
    Wm1g = Wm1 * g_low[None, :]
    Walg = Wal * g_low[None, :]
    bb = (b_low + b_high) * 0.5
    xa_bias = Wal @ bb
    ones128 = np.ones((128, 128), f32)
    ident = np.eye(128, dtype=f32)

    def pf(x):  # [k, ...] stacked lhsT tiles -> partition-first
        return np.ascontiguousarray(np.moveaxis(x, 1, 0)) if False else x

    wm1_h = np.ascontiguousarray(
        np.stack([Wm1g.T[0:128, 0:128], Wm1g.T[128:256, 128:256]]).transpose(1, 0, 2)).astype(bf16)
    wm2_h = np.ascontiguousarray(
        np.stack([w_ml2.T[0:128], w_ml2.T[128:256]]).transpose(1, 0, 2)).astype(bf16)
    wal_h = np.ascontiguousarray(
        np.stack([Walg.T[0:128, 0:128], Walg.T[128:256, 128:256]]).transpose(1, 0, 2)).astype(bf16)

    s1_core = []
    for core in range(NCORES):
        b, hf = core // 2, core % 2
        r0 = hf * R
        s1_core.append(dict(
            lo=_chunk(low, b, r0, r0 + R, 0, 0).astype(bf16),
            hi=_chunk(high, b, r0, r0 + R, 0, 0).astype(bf16),
            ones=ones128.astype(bf16), ident=ident.astype(bf16),
            wm1=wm1_h, wm2=wm2_h, wal=wal_h,
        ))

    global _last_s1_inputs
    _last_s1_inputs = s1_core
    if _S1 is None:
        _S1 = build_stage1()
    res1 = run_bass_kernel_spmd(_S1, s1_core, core_ids=list(range(NCORES)))

    S = np.zeros((B, NCL, 256), f32)
    Z = np.zeros((B, NCL), f32)
    for core in range(NCORES):
        b = core // 2
        S[b] += res1.results[core]["S_out"]
        Z[b] += res1.results[core]["Z_out"][:, 0]
    cf = S / Z[:, :, None] + xa_bias[None, None, :]
    cf = (1.0 - MOM) * cf + MOM * memory
    kv = cf @ w_kv.T + b_kv
    k, v = kv[:, :, :256], kv[:, :, 256:]

    # folded q-path biases -> per (b, head, class) logit bias
    cb1 = b_low * w_q_dw[:, 0].sum(axis=(1, 2)) + b_q_dw
    cb2 = w_q_pw @ cb1 + b_q_pw
    lbh = np.zeros((B, NH, NCL), f32)
    for h in range(NH):
        lbh[:, h, :] = np.einsum("bnd,d->bn", k[:, :, 32 * h:32 * h + 32],
                                 cb2[32 * h:32 * h + 32])

    wqdw_diag = np.zeros((2, 9, 128, 128), f32)
    wdw_g = w_q_dw[:, 0] * g_low[:, None, None]
    for ct in range(2):
        for tap in range(9):
            kh, kw = tap // 3, tap % 3
            np.fill_diagonal(wqdw_diag[ct, tap], wdw_g[ct * 128:(ct + 1) * 128, kh, kw])
    wdwm_diag = np.zeros((8, 9, 128, 128), f32)
    for g in range(8):
        for tap in range(9):
            kh, kw = tap // 3, tap % 3
            np.fill_diagonal(wdwm_diag[g, tap], w_mlp_dw[g * 128:(g + 1) * 128, 0, kh, kw])
    W1g = w_mlp1 * g_mlp[None, :]
    b1v = b_mlp1 + w_mlp1 @ b_mlp

    def lhsT_tiles(Wt, nk):  # W [out, in] -> [128, nk, out] partition-first lhsT
        st = np.stack([Wt.T[i * 128:(i + 1) * 128] for i in range(nk)])
        return np.ascontiguousarray(st.transpose(1, 0, 2)).astype(bf16)

    wqpw_h = lhsT_tiles(w_q_pw, 2)
    wproj_h = lhsT_tiles(w_proj, 2)
    wmlp1_h = lhsT_tiles(W1g, 2)
    wmlp2_h = lhsT_tiles(w_mlp2, 8)
    wqdw_h = np.ascontiguousarray(wqdw_diag.transpose(2, 0, 1, 3)).astype(bf16)
    wdwm_h = np.ascontiguousarray(wdwm_diag.transpose(2, 0, 1, 3)).astype(bf16)

    s2_core = []
    for core in range(NCORES):
        b, hf = core // 2, core % 2
        r0 = hf * R
        lo_ch = _chunk(low, b, r0, r0 + R, 2, 2)
        kbd = np.zeros((2, 128, 152), f32)
        vbd = np.zeros((128, 256), f32)
        onesbd = np.zeros((128, 4), f32)
        expd = np.zeros((4, 76), f32)
        bexp = np.zeros((128, 2), f32)
        for h in range(NH):
            hf2 = h // 4
            base = (h % 4) * NCL
            j = hf2 * 76 + base
            d0 = 32 * h
            for n in range(NCL):
                kt0, p0 = divmod(d0, 128)
                kbd[kt0, p0:p0 + 32, j + n] = k[b, n, d0:d0 + 32]
                vbd[base + n, hf2 * 128 + (d0 % 128):hf2 * 128 + (d0 % 128) + 32] = \
                    v[b, n, d0:d0 + 32]
            onesbd[base:base + NCL, h % 4] = 1.0
            expd[h % 4, base:base + NCL] = 1.0
            bexp[base:base + NCL, hf2] = -SCALE * lbh[b, h, :]

        zmask = np.ones((128, 2, TN), f32)
        if hf == 0:
            zmask[:, 0, 0:256] = 0.0      # tile 0: image rows -2, -1
        else:
            zmask[:, 1, 256:512] = 0.0    # tile 16: image rows 128, 129

        s2_core.append(dict(
            lo16=lo_ch.astype(bf16),
            ones=ones128.astype(bf16),
            kbd=np.ascontiguousarray(kbd.transpose(1, 0, 2)).astype(bf16),
            vbd=vbd.astype(bf16), onesbd=onesbd.astype(bf16),
            expd=expd.astype(f32), bexp=bexp.astype(f32),
            wqdw=wqdw_h, wqpw=wqpw_h, wproj=wproj_h,
            wmlp1=wmlp1_h, wdwm=wdwm_h, wmlp2=wmlp2_h,
            bprj=np.ascontiguousarray(b_proj.reshape(2, 128).T).astype(f32),
            b1=np.ascontiguousarray(b1v.reshape(8, 128).T).astype(f32),
            bdw=np.ascontiguousarray(b_mlp_dw.reshape(8, 128).T).astype(f32),
            b2=np.ascontiguousarray(b_mlp2.reshape(2, 128).T).astype(f32),
            zmask=zmask.astype(bf16),
        ))

    global _last_s2_inputs
    _last_s2_inputs = s2_core
    if _S2 is None:
        _S2 = build_stage2()
    res2 = run_bass_kernel_spmd(_S2, s2_core, core_ids=list(range(NCORES)))

    out = np.zeros((B, C, H, W), np.float32)
    for core in range(NCORES):
        b, hf = core // 2, core % 2
        o = res2.results[core]["OUT"]            # [128, 2, 8192]
        o = o.transpose(1, 0, 2).reshape(C, R, W)
        out[b, :, hf * R:(hf + 1) * R, :] = o
    return out



# revision 29
# speedup vs baseline: 1.3947x; 1.3947x over previous
"""CPGA Trainium2 Bass kernel.

Two SPMD launches over 8 NeuronCores, sharded (batch, row-half):
  stage 1: LN(low)/LN(high) -> fused -> mask logits + aligned features ->
           per-core partial class-prototype sums (streaming spatial softmax).
  host   : combine partials -> cf -> memory mix -> k/v + folded biases.
  stage 2: query conv path, cross-attention over 19 prototypes, proj+residual,
           LN, conv-FFN (1x1 -> depthwise 3x3 -> gelu -> 1x1), final residual.

Layout: channels on partitions, pixels on the free dim (all SBUF tiles are
partition-first). Depthwise 3x3 convs run on TensorE as 9 diagonal matmuls
with shifted rhs slices over halo-padded tiles. LayerNorm stats use
ones-matmul partition broadcasts; rstd = exp(-0.5*ln(var+eps)).
"""

import numpy as np
import ml_dtypes

import concourse.bass as bass
import concourse.mybir as mybir
from concourse import bacc
from concourse.tile import TileContext
from concourse.bass_utils import run_bass_kernel_spmd

# Prefer the combined Ln+Exp activation table so per-tile Ln/Exp/Square
# sequences don't thrash ACT_TABLE_LOADs (the insertion pass picks the
# first table containing each func).
from concourse import hw_specs as _hw_specs
_orig_get_act_tables = _hw_specs.get_activation_tables

def _act_tables_combined_first(arch):
    # Keep canonical order/indices (walrus maps set-id by index); advertise
    # only the two tables we want selected so the greedy pass never picks a
    # set lacking a func we'll need next (Ln+Exp+Square live together in
    # natural_log_exp_and_others; Gelu needs gelu_and_others).
    tabs = dict(_orig_get_act_tables(arch))
    keep = ("natural_log_exp_and_others", "gelu_and_others")
    return {k: (v if k in keep else type(v)()) for k, v in tabs.items()}

bacc.get_activation_tables = _act_tables_combined_first

BF = mybir.dt.bfloat16
F32 = mybir.dt.float32
F32R = mybir.dt.float32r
AL = mybir.AluOpType
AF = mybir.ActivationFunctionType
bf16 = ml_dtypes.bfloat16

B, C, H, W = 4, 256, 128, 128
NCL, NH, HD = 19, 8, 32
SCALE = HD ** -0.5
MOM = 0.1
EPS = 1e-5
NCORES = 8
R = 64            # rows per core chunk
S1_T = 16         # stage-1 tiles of 512 px (64 rows)
S2_T = 17         # stage-2 tiles (68 rows incl 2-row halo each side)
TN = 512          # pixels per tile
QH = 130          # free-dim halo for dw conv tiles (even -> 4x DVE copies)
LN2 = float(np.log(0.5))

GP_APPLY = True   # put the LN-apply multiplies on GPSIMD
TAP_ORDER = [(dr, 0) for dr in (-1, 0, 1)] + [(dr, dc) for dc in (-1, 1) for dr in (-1, 0, 1)]


# ----------------------------------------------------------------------------
# stage 1 builder
# ----------------------------------------------------------------------------

def build_stage1():
    """Transpose-free stage 1: class-prototype sums via direct px-partition
    matmuls. Per 128-px block B: mk^T[px,19] = sum_ct c1a[:,ct,B]^T @ wm2;
    e^T = exp(mk^T); xa^T[px,256] = sum_ct f[:,ct,B]^T @ WalgT; then
    S += e^T.T @ xa^T and Z += e^T.T @ 1 accumulate in PSUM across all
    blocks. No TensorE transposes, no spatial-softmax accumulator reads.
    """
    nc = bacc.Bacc()
    lo = nc.dram_tensor("lo", [128, 2, S1_T * TN], BF, kind="ExternalInput")
    hi = nc.dram_tensor("hi", [128, 2, S1_T * TN], BF, kind="ExternalInput")
    ones = nc.dram_tensor("ones", [128, 128], BF, kind="ExternalInput")
    wm1 = nc.dram_tensor("wm1", [128, 2, 128], BF, kind="ExternalInput")
    wm2 = nc.dram_tensor("wm2", [128, 2, NCL], BF, kind="ExternalInput")
    walT = nc.dram_tensor("walT", [128, 2, 256], BF, kind="ExternalInput")
    S_out = nc.dram_tensor("S_out", [NCL, 256], F32, kind="ExternalOutput")
    Z_out = nc.dram_tensor("Z_out", [NCL, 1], F32, kind="ExternalOutput")

    with TileContext(nc) as tc:
        with (
            tc.tile_pool(name="cst", bufs=1) as cst,
            tc.tile_pool(name="sb", bufs=6) as sb,
            tc.tile_pool(name="st", bufs=4) as st,
            tc.tile_pool(name="ps_a", bufs=1, space="PSUM") as ps_a,
            tc.tile_pool(name="ps_b", bufs=2, space="PSUM") as ps_b,
        ):
            ones_t = cst.tile([128, 128], BF, tag="ones")
            nc.sync.dma_start(ones_t[:], ones[:])
            wm1_t = cst.tile([128, 2, 128], BF, tag="wm1")
            nc.sync.dma_start(wm1_t[:], wm1[:])
            wm2_t = cst.tile([128, 2, NCL], BF, tag="wm2")
            nc.sync.dma_start(wm2_t[:], wm2[:])
            walT_t = cst.tile([128, 2, 256], BF, tag="walT")
            nc.sync.dma_start(walT_t[:], walT[:])
            cfz = ps_a.tile([NCL, 512], F32, tag="cf")
            cf = cfz[:, 0:256]
            zps = cfz[:, 256:257]
            epsb = cst.tile([128, 1], F32, tag="epsb")
            nc.vector.memset(epsb[:], EPS)
            ln2b = cst.tile([128, 1], F32, tag="ln2b")
            nc.vector.memset(ln2b[:], LN2)

            for t in range(S1_T):
                sl = slice(t * TN, (t + 1) * TN)
                lo_t = sb.tile([128, 2, TN], BF, tag="lo")
                nc.sync.dma_start(lo_t[:], lo[:, :, sl])
                hi_t = sb.tile([128, 2, TN], BF, tag="hi")
                nc.sync.dma_start(hi_t[:], hi[:, :, sl])

                sql = sb.tile([128, 2, TN], BF, tag="sql")
                sqh = sb.tile([128, 2, TN], BF, tag="sqh")
                for ct in range(2):
                    nc.gpsimd.tensor_tensor(sql[:, ct, :], lo_t[:, ct, :], lo_t[:, ct, :], op=AL.mult)
                    nc.vector.tensor_tensor(sqh[:, ct, :], hi_t[:, ct, :], hi_t[:, ct, :], op=AL.mult)

                def ln_stats(x_t, sq_t, tag):
                    s1 = ps_a.tile([128, TN], F32, tag="s1", bufs=2)
                    nc.tensor.matmul(s1[:], ones_t[:], x_t[:, 0, :], start=True, stop=False)
                    nc.tensor.matmul(s1[:], ones_t[:], x_t[:, 1, :], start=False, stop=True)
                    s2 = ps_a.tile([128, TN], F32, tag="s2", bufs=2)
                    nc.tensor.matmul(s2[:], ones_t[:], sq_t[:, 0, :], start=True, stop=False)
                    nc.tensor.matmul(s2[:], ones_t[:], sq_t[:, 1, :], start=False, stop=True)
                    mu2 = st.tile([128, TN], F32, tag="mu2" + tag)
                    nc.scalar.activation(mu2[:], s1[:], AF.Square, scale=1.0 / C)
                    var = st.tile([128, TN], F32, tag="var" + tag)
                    nc.vector.scalar_tensor_tensor(var[:], s2[:], 1.0 / C, mu2[:],
                                                   op0=AL.mult, op1=AL.subtract)
                    nc.scalar.activation(var[:], var[:], AF.Ln, bias=epsb[:])
                    r2 = st.tile([128, TN], BF, tag="r2" + tag)   # 0.5 * rstd
                    nc.scalar.activation(r2[:], var[:], AF.Exp, scale=-0.5, bias=ln2b[:])
                    m2 = st.tile([128, TN], BF, tag="m2" + tag)   # 0.5 * mu * rstd
                    nc.vector.scalar_tensor_tensor(m2[:], s1[:], 1.0 / C, r2[:],
                                                   op0=AL.mult, op1=AL.mult)
                    return r2, m2

                rl2, m2l = ln_stats(lo_t, sql, "l")
                rh2, m2h = ln_stats(hi_t, sqh, "h")
                m12 = st.tile([128, TN], BF, tag="m12")
                nc.vector.tensor_tensor(m12[:], m2l[:], m2h[:], op=AL.add)

                # fused = low*rl2 + high*rh2 - m12
                f_t = sb.tile([128, 2, TN], BF, tag="f")
                t1 = sb.tile([128, 2, TN], BF, tag="t1")
                t2 = sb.tile([128, 2, TN], BF, tag="t2")
                for ct in range(2):
                    nc.gpsimd.tensor_tensor(t1[:, ct, :], lo_t[:, ct, :], rl2[:], op=AL.mult)
                    nc.vector.tensor_tensor(t2[:, ct, :], hi_t[:, ct, :], rh2[:], op=AL.mult)
                for ct in range(2):
                    nc.vector.tensor_tensor(f_t[:, ct, :], t1[:, ct, :], t2[:, ct, :], op=AL.add)
                    nc.vector.tensor_tensor(f_t[:, ct, :], f_t[:, ct, :], m12[:], op=AL.subtract)

                # mask conv1 (block-diag grouped)
                c1a = sb.tile([128, 2, TN], BF, tag="c1a")
                for ct in range(2):
                    c1p = ps_b.tile([128, TN], F32, tag="mmx", name="c1p", bufs=3)
                    nc.tensor.matmul(c1p[:], wm1_t[:, ct, :], f_t[:, ct, :], start=True, stop=True)
                    if ct == 0:
                        nc.scalar.copy(c1a[:, ct, :], c1p[:])
                    else:
                        nc.vector.tensor_copy(c1a[:, ct, :], c1p[:])

                # per 128-px block: mk^T -> e^T; xa^T; accumulate S and Z
                for blk in range(4):
                    bsl = slice(blk * 128, (blk + 1) * 128)
                    mkp = ps_b.tile([128, NCL], F32, tag="mmx", name="mkp", bufs=3)
                    for ct in range(2):
                        nc.tensor.matmul(mkp[:], c1a[:, ct, bsl], wm2_t[:, ct, :],
                                         start=(ct == 0), stop=(ct == 1))
                    eT = sb.tile([128, NCL], BF, tag="eT")
                    nc.scalar.activation(eT[:], mkp[:], AF.Exp)
                    xap = ps_b.tile([128, 256], F32, tag="mmx", name="xap", bufs=3)
                    for ct in range(2):
                        nc.tensor.matmul(xap[:], f_t[:, ct, bsl], walT_t[:, ct, :],
                                         start=(ct == 0), stop=(ct == 1))
                    xaTs = sb.tile([128, 256], BF, tag="xaTs")
                    nc.scalar.copy(xaTs[:], xap[:])
                    first = (t == 0 and blk == 0)
                    last = (t == S1_T - 1 and blk == 3)
                    nc.tensor.matmul(cfz[:, 0:256], eT[:], xaTs[:], start=first, stop=last)
                    nc.tensor.matmul(cfz[:, 256:257], eT[:], ones_t[:, 0:1], start=first, stop=last)

            S_sb = cst.tile([NCL, 256], F32, tag="S_sb")
            nc.vector.tensor_copy(S_sb[:], cfz[:, 0:256])
            nc.sync.dma_start(S_out[:], S_sb[:])
            z_sb = cst.tile([NCL, 1], F32, tag="z_sb")
            nc.vector.tensor_copy(z_sb[:], cfz[:, 256:257])
            nc.sync.dma_start(Z_out[:], z_sb[:])

    nc.finalize()
    return nc


# ----------------------------------------------------------------------------
# stage 2 builder
# ----------------------------------------------------------------------------

def build_stage2():
    """Fully fused stage 2: one software-pipelined loop per tile.

    iter t: build_query(t) | attn(t-1) | build_z(t-2) | ffn(t-3).
    Per-tile LN stats (ones-matmul broadcast + Ln/Exp from the combined act
    table); out/yl kept as rotating bf16 SBUF tiles; FFN matmuls interleave
    with the attention chain so TensorE never idles past the HAM window.
    """
    nc = bacc.Bacc()
    NPX = S2_T * TN
    lo16 = nc.dram_tensor("lo16", [128, 2, NPX], BF, kind="ExternalInput")
    ones = nc.dram_tensor("ones", [128, 128], BF, kind="ExternalInput")
    kbd = nc.dram_tensor("kbd", [128, 2, 152], BF, kind="ExternalInput")
    pvbd = nc.dram_tensor("pvbd", [128, 2, 256], BF, kind="ExternalInput")
    onesbd = nc.dram_tensor("onesbd", [128, 4], BF, kind="ExternalInput")
    expd = nc.dram_tensor("expd", [4, 76], F32, kind="ExternalInput")
    bexp = nc.dram_tensor("bexp", [128, 2], F32, kind="ExternalInput")
    wqdw = nc.dram_tensor("wqdw", [128, 2, 9, 128], BF, kind="ExternalInput")
    wmlp1 = nc.dram_tensor("wmlp1", [128, 2, 1024], BF, kind="ExternalInput")
    wdwm = nc.dram_tensor("wdwm", [128, 8, 9, 128], BF, kind="ExternalInput")
    wmlp2 = nc.dram_tensor("wmlp2", [128, 8, 256], BF, kind="ExternalInput")
    bprj = nc.dram_tensor("bprj", [128, 2], F32, kind="ExternalInput")
    b1 = nc.dram_tensor("b1", [128, 8], F32, kind="ExternalInput")
    bdw = nc.dram_tensor("bdw", [128, 8], F32, kind="ExternalInput")
    b2 = nc.dram_tensor("b2", [128, 2], F32, kind="ExternalInput")
    zmask = nc.dram_tensor("zmask", [128, 2, TN], BF, kind="ExternalInput")
    OUT = nc.dram_tensor("OUT", [128, 2, S1_T * TN], F32, kind="ExternalOutput")

    with TileContext(nc) as tc:
        with (
            tc.tile_pool(name="cst", bufs=1) as cst,
            tc.tile_pool(name="lop", bufs=5) as lop,
            tc.tile_pool(name="sb2", bufs=3) as sb2,
            tc.tile_pool(name="qp", bufs=4) as qp,
            tc.tile_pool(name="zp", bufs=4) as zp,
            tc.tile_pool(name="outp", bufs=5) as outp,
            tc.tile_pool(name="ylp", bufs=3) as ylp,
            tc.tile_pool(name="ps", bufs=2, space="PSUM") as ps,
        ):
            ones_t = cst.tile([128, 128], BF, tag="ones"); nc.sync.dma_start(ones_t[:], ones[:])
            kbd_t = cst.tile([128, 2, 152], BF, tag="kbd"); nc.sync.dma_start(kbd_t[:], kbd[:])
            pvbd_t = cst.tile([128, 2, 256], BF, tag="pvbd"); nc.sync.dma_start(pvbd_t[:], pvbd[:])
            obd_t = cst.tile([128, 4], BF, tag="obd"); nc.sync.dma_start(obd_t[:], onesbd[:])
            expd_t = cst.tile([4, 76], F32, tag="expd"); nc.sync.dma_start(expd_t[:], expd[:])
            bexp_t = cst.tile([128, 2], F32, tag="bexp"); nc.sync.dma_start(bexp_t[:], bexp[:])
            wqdw_t = cst.tile([128, 2, 9, 128], BF, tag="wqdw"); nc.sync.dma_start(wqdw_t[:], wqdw[:])
            wmlp1_t = cst.tile([128, 2, 1024], BF, tag="wmlp1"); nc.sync.dma_start(wmlp1_t[:], wmlp1[:])
            wdwm_t = cst.tile([128, 8, 9, 128], BF, tag="wdwm"); nc.sync.dma_start(wdwm_t[:], wdwm[:])
            wmlp2_t = cst.tile([128, 8, 256], BF, tag="wmlp2"); nc.sync.dma_start(wmlp2_t[:], wmlp2[:])
            bprj_t = cst.tile([128, 2], F32, tag="bprj"); nc.sync.dma_start(bprj_t[:], bprj[:])
            b1_t = cst.tile([128, 8], F32, tag="b1"); nc.sync.dma_start(b1_t[:], b1[:])
            bdw_t = cst.tile([128, 8], F32, tag="bdw"); nc.sync.dma_start(bdw_t[:], bdw[:])
            b2_t = cst.tile([128, 2], F32, tag="b2"); nc.sync.dma_start(b2_t[:], b2[:])
            zm_t = cst.tile([128, 2, TN], BF, tag="zm"); nc.sync.dma_start(zm_t[:], zmask[:])
            epsb = cst.tile([128, 1], F32, tag="epsb")
            nc.vector.memset(epsb[:], EPS)

            qtiles = {}
            ztiles = {}
            lo_a = {}
            out_a = {}
            yl_a = {}
            e_a = {}
            rz_a = {}
            qsb_a = {}
            f01_a = {}
            qd_a = {}
            en_a = {}
            sq_a = {}

            # per-tile LN stats -> (rstd, mu*rstd) bf16 full-width tiles
            def ln_tile(x0, x1, nm, sq=None):
                if sq is None:
                    sq = sb2.tile([128, 2, TN], BF, tag="sq", name="sq" + nm, bufs=3)
                    nc.gpsimd.tensor_tensor(sq[:, 0, :], x0, x0, op=AL.mult)
                    nc.gpsimd.tensor_tensor(sq[:, 1, :], x1, x1, op=AL.mult)
                s1 = ps.tile([128, TN], F32, tag="st", name="s1" + nm)
                nc.tensor.matmul(s1[:], ones_t[:], x0, start=True, stop=False)
                nc.tensor.matmul(s1[:], ones_t[:], x1, start=False, stop=True)
                s2 = ps.tile([128, TN], F32, tag="st", name="s2" + nm)
                nc.tensor.matmul(s2[:], ones_t[:], sq[:, 0, :], start=True, stop=False)
                nc.tensor.matmul(s2[:], ones_t[:], sq[:, 1, :], start=False, stop=True)
                mu2 = sb2.tile([128, TN], BF, tag="mu2", name="mu2" + nm)
                nc.scalar.activation(mu2[:], s1[:], AF.Square, scale=1.0 / C)
                mu_b = sb2.tile([128, TN], BF, tag="mu_b", name="mu_b" + nm)
                nc.scalar.activation(mu_b[:], s1[:], AF.Identity, scale=1.0 / C)
                var = sb2.tile([128, TN], F32, tag="var", name="var" + nm)
                nc.vector.scalar_tensor_tensor(var[:], s2[:], 1.0 / C, mu2[:],
                                               op0=AL.mult, op1=AL.subtract)
                nc.scalar.activation(var[:], var[:], AF.Ln, bias=epsb[:])
                rl = sb2.tile([128, TN], BF, tag="rl", name="rl" + nm)
                nc.scalar.activation(rl[:], var[:], AF.Exp, scale=-0.5)
                m2 = sb2.tile([128, TN], BF, tag="m2", name="m2" + nm)
                nc.vector.tensor_tensor(m2[:], mu_b[:], rl[:], op=AL.mult)
                return rl, m2

            def bq_dma(t):
                sl = slice(t * TN, (t + 1) * TN)
                lo_t = lop.tile([128, 2, TN], BF, tag="lo", name="lo_t")
                nc.sync.dma_start(lo_t[:], lo16[:, :, sl])
                lo_a[t] = lo_t

            def bq_sq(t):
                lo_t = lo_a[t]
                sq = sb2.tile([128, 2, TN], BF, tag="sq", name="sqq", bufs=3)
                nc.gpsimd.tensor_tensor(sq[:, 0, :], lo_t[:, 0, :], lo_t[:, 0, :], op=AL.mult)
                nc.gpsimd.tensor_tensor(sq[:, 1, :], lo_t[:, 1, :], lo_t[:, 1, :], op=AL.mult)
                sq_a[t] = sq

            def build_query(t):
                lo_t = lo_a[t]
                rl, m2 = ln_tile(lo_t[:, 0, :], lo_t[:, 1, :], "q", sq=sq_a.pop(t))
                qt = qp.tile([128, 2, 2 * QH + TN], BF, tag="qt")
                qtiles[t] = qt
                for ct in range(2):
                    nc.gpsimd.tensor_tensor(qt[:, ct, QH:QH + TN], lo_t[:, ct, :], rl[:],
                                            op=AL.mult)
                for ct in range(2):
                    nc.vector.tensor_tensor(qt[:, ct, QH:QH + TN], qt[:, ct, QH:QH + TN],
                                            m2[:], op=AL.subtract)
                if t == 0:
                    nc.vector.memset(qt[:, :, 0:QH], 0.0)
                else:
                    for ct in range(2):
                        nc.vector.tensor_copy(qt[:, ct, 0:QH],
                                              qtiles[t - 1][:, ct, TN:TN + QH])
                        nc.vector.tensor_copy(qtiles[t - 1][:, ct, QH + TN:],
                                              qt[:, ct, QH:2 * QH])
                if t == S2_T - 1:
                    nc.vector.memset(qt[:, :, QH + TN:], 0.0)

            def dw9(psum, wtile, src):
                for ti, (dr, dc) in enumerate(TAP_ORDER):
                    tap = (dr + 1) * 3 + (dc + 1)
                    off0 = QH + dr * 128
                    lhs = wtile[:, tap, :]
                    if dc == 0:
                        nc.tensor.matmul(psum[:], lhs, src[:, off0:off0 + TN],
                                         start=(ti == 0), stop=(ti == 8))
                    else:
                        rhs3 = src[:, off0:off0 + TN].rearrange("p (r w) -> p r w", w=128)
                        out3 = psum[:].rearrange("p (r w) -> p r w", w=128)
                        if dc == -1:
                            nc.tensor.matmul(out3[:, :, 1:128], lhs, rhs3[:, :, 0:127],
                                             start=False, stop=(ti == 8))
                        else:
                            nc.tensor.matmul(out3[:, :, 0:127], lhs, rhs3[:, :, 1:128],
                                             start=False, stop=(ti == 8))

            def attn_qdw(s, ct):
                qt = qtiles[s]
                if ct == 0:
                    qd = sb2.tile([128, 2, TN], BF, tag="qd")
                    qd_a[s] = qd
                qd = qd_a[s]
                qdp = ps.tile([128, TN], F32, tag="mm", name="qdp", bufs=3)
                dw9(qdp, wqdw_t[:, ct], qt[:, ct, :])
                nc.vector.tensor_copy(qd[:, ct, :], qdp[:])

            def attn_qk2(s, hf):
                qd = qd_a[s]
                if hf == 0:
                    e_a[s] = []
                    rz_a[s] = sb2.tile([4, 2, TN], F32, tag="rz", bufs=2, name="rz")
                lp = ps.tile([128, TN], F32, tag="at", name="lp", bufs=1)
                for kt in range(2):
                    nc.tensor.matmul(lp[0:76, :], kbd_t[:, kt, hf * 76:hf * 76 + 76],
                                     qd[:, kt, :], start=(kt == 0), stop=(kt == 1))
                e_h = sb2.tile([76, TN], BF, tag="eh%d" % hf)
                nc.scalar.activation(e_h[:], lp[0:76, :], AF.Exp, scale=-SCALE,
                                     bias=bexp_t[0:76, hf:hf + 1])
                e_a[s].append(e_h)
                zp_ = ps.tile([4, TN], F32, tag="at", name="zp_", bufs=1)
                nc.tensor.matmul(zp_[:], obd_t[0:76, :], e_h[:], start=True, stop=True)
                nc.vector.reciprocal_approx_fast(rz_a[s][:, hf, :], zp_[:])
                if hf == 1:
                    del qd_a[s]

            def attn_en(s, hf):
                e_ab = e_a[s]
                rz = rz_a[s]
                rzx = ps.tile([128, TN], F32, tag="at", name="rzx", bufs=1)
                nc.tensor.matmul(rzx[0:76, :], expd_t[:], rz[:, hf, :],
                                 start=True, stop=True)
                en = sb2.tile([76, TN], BF, tag="en%d" % hf, name="en")
                nc.vector.tensor_tensor(en[:], e_ab[hf][:], rzx[0:76, :], op=AL.mult)
                en_a.setdefault(s, []).append(en)
                if hf == 1:
                    del e_a[s]
                    del rz_a[s]

            def attn_proj(s, mt):
                en = en_a[s]
                if mt == 0:
                    out_t = outp.tile([128, 2, TN], BF, tag="out")
                    out_a[s] = out_t
                out_t = out_a[s]
                op_ = ps.tile([128, TN], F32, tag="mm", name="op_", bufs=3)
                for hf in range(2):
                    nc.tensor.matmul(op_[:], pvbd_t[0:76, hf, mt * 128:(mt + 1) * 128],
                                     en[hf][:], start=(hf == 0), stop=(hf == 1))
                nc.vector.scalar_tensor_tensor(out_t[:, mt, :], op_[:],
                                               bprj_t[:, mt:mt + 1],
                                               lo_a[s][:, mt, :], op0=AL.add, op1=AL.add)
                if mt == 1:
                    del en_a[s]
                    del lo_a[s]

            def attn_ln(s):
                out_t = out_a[s]
                ro, m2o = ln_tile(out_t[:, 0, :], out_t[:, 1, :], "o")
                yl_t = ylp.tile([128, 2, TN], BF, tag="yl")
                yl_a[s] = yl_t
                for ct in range(2):
                    nc.gpsimd.tensor_tensor(yl_t[:, ct, :], out_t[:, ct, :], ro[:],
                                            op=AL.mult)
                for ct in range(2):
                    nc.vector.tensor_tensor(yl_t[:, ct, :], yl_t[:, ct, :], m2o[:],
                                            op=AL.subtract)

            def build_z_pre(t):
                zt = zp.tile([128, 8, 2 * QH + TN], BF, tag="zt")
                ztiles[t] = zt

            def build_z_g(t, g):
                yl_t = yl_a[t]
                zt = ztiles[t]
                m1p = ps.tile([128, TN], F32, tag="mm", name="m1p", bufs=3)
                for kt in range(2):
                    nc.tensor.matmul(m1p[:], wmlp1_t[:, kt, g * 128:(g + 1) * 128],
                                     yl_t[:, kt, :], start=(kt == 0), stop=(kt == 1))
                if g % 2 == 0:
                    nc.scalar.activation(zt[:, g, QH:QH + TN], m1p[:], AF.Identity,
                                         bias=b1_t[:, g:g + 1])
                else:
                    nc.vector.tensor_scalar(zt[:, g, QH:QH + TN], m1p[:],
                                            b1_t[:, g:g + 1], None, op0=AL.add)
                if t == 0:
                    nc.vector.tensor_tensor(zt[:, g, QH:QH + TN], zt[:, g, QH:QH + TN],
                                            zm_t[:, 0, :], op=AL.mult)
                elif t == S2_T - 1:
                    nc.vector.tensor_tensor(zt[:, g, QH:QH + TN], zt[:, g, QH:QH + TN],
                                            zm_t[:, 1, :], op=AL.mult)

            def build_z_post(t):
                zt = ztiles[t]
                del yl_a[t]
                if t == 0:
                    nc.vector.memset(zt[:, :, 0:QH], 0.0)
                else:
                    for g in range(8):
                        nc.vector.tensor_copy(zt[:, g, 0:QH],
                                              ztiles[t - 1][:, g, TN:TN + QH])
                        nc.vector.tensor_copy(ztiles[t - 1][:, g, QH + TN:],
                                              zt[:, g, QH:2 * QH])
                if t == S2_T - 1:
                    nc.vector.memset(zt[:, :, QH + TN:], 0.0)

            def ffn_g(s, g):
                zt = ztiles[s]
                if g == 0:
                    f0 = ps.tile([128, TN], F32, tag="f01", name="f0")
                    f1 = ps.tile([128, TN], F32, tag="f01", name="f1")
                    f01_a[s] = (f0, f1)
                f0, f1 = f01_a[s]
                dwp = ps.tile([128, TN], F32, tag="mm", name="dwp", bufs=3)
                dw9(dwp, wdwm_t[:, g], zt[:, g, :])
                gel = sb2.tile([128, TN], BF, tag="gel")
                nc.scalar.activation(gel[:], dwp[:], AF.Gelu, bias=bdw_t[:, g:g + 1])
                nc.tensor.matmul(f0[:], wmlp2_t[:, g, 0:128], gel[:],
                                 start=(g == 0), stop=(g == 7))
                nc.tensor.matmul(f1[:], wmlp2_t[:, g, 128:256], gel[:],
                                 start=(g == 0), stop=(g == 7))

            def ffn_fin(s):
                f0, f1 = f01_a.pop(s)
                if s == 0:
                    px0, px1, o0 = 256, TN, 0
                elif s == S2_T - 1:
                    px0, px1, o0 = 0, 256, (S2_T - 1) * TN - 256
                else:
                    px0, px1, o0 = 0, TN, s * TN - 256
                n = px1 - px0
                for ct, fps in enumerate((f0, f1)):
                    fin = sb2.tile([128, TN], F32, tag="fin", name="fin")
                    nc.vector.scalar_tensor_tensor(fin[:, 0:n], fps[:, px0:px1],
                                                   b2_t[:, ct:ct + 1],
                                                   out_a[s][:, ct, px0:px1],
                                                   op0=AL.add, op1=AL.add)
                    nc.sync.dma_start(OUT[:, ct, o0:o0 + n], fin[:, 0:n])
                del out_a[s]

            # software pipeline: z(t-3) | attn(t-2) | ffn(t-4) | query(t),
            # coarse chunks so each engine queue drains in dependency order,
            # ffn dw groups filling the PE while attention chains run.
            for t in range(S2_T + 4):
                bz = 3 <= t < S2_T + 3      # build_z(t-3)
                qk = 2 <= t < S2_T + 2      # attn(t-2)
                fn = 4 <= t < S2_T + 4      # ffn(t-4)
                if bz:
                    build_z_pre(t - 3)
                    for g in range(8):
                        build_z_g(t - 3, g)
                    build_z_post(t - 3)
                if qk:
                    attn_qdw(t - 2, 0)
                    attn_qdw(t - 2, 1)
                    # both attention Exps back-to-back: exactly one act-table
                    # switch into Gelu below and one back per iteration
                    attn_qk2(t - 2, 0)
                    attn_qk2(t - 2, 1)
                if fn:
                    for g in range(8):
                        ffn_g(t - 4, g)
                if qk:
                    attn_en(t - 2, 0)
                    attn_en(t - 2, 1)
                    attn_proj(t - 2, 0)
                    attn_proj(t - 2, 1)
                if fn:
                    ffn_fin(t - 4)
                    del ztiles[t - 4]
                if qk:
                    attn_ln(t - 2)
                if t < S2_T:
                    build_query(t)
                if qk:
                    del qtiles[t - 2]

    nc.finalize()
    return nc


# ----------------------------------------------------------------------------
# stage 2 builder
# ----------------------------------------------------------------------------

def build_stage2():
    """Fully fused stage 2: one software-pipelined loop per tile.

    iter t: build_query(t) | attn(t-1) | build_z(t-2) | ffn(t-3).
    Per-tile LN stats (ones-matmul broadcast + Ln/Exp from the combined act
    table); out/yl kept as rotating bf16 SBUF tiles; FFN matmuls interleave
    with the attention chain so TensorE never idles past the HAM window.
    """
    nc = bacc.Bacc()
    NPX = S2_T * TN
    lo16 = nc.dram_tensor("lo16", [128, 2, NPX], BF, kind="ExternalInput")
    ones = nc.dram_tensor("ones", [128, 128], BF, kind="ExternalInput")
    kbd = nc.dram_tensor("kbd", [128, 2, 152], BF, kind="ExternalInput")
    pvbd = nc.dram_tensor("pvbd", [128, 2, 256], BF, kind="ExternalInput")
    onesbd = nc.dram_tensor("onesbd", [128, 4], BF, kind="ExternalInput")
    expd = nc.dram_tensor("expd", [4, 76], F32, kind="ExternalInput")
    bexp = nc.dram_tensor("bexp", [128, 2], F32, kind="ExternalInput")
    wqdw = nc.dram_tensor("wqdw", [128, 2, 9, 128], BF, kind="ExternalInput")
    wmlp1 = nc.dram_tensor("wmlp1", [128, 2, 1024], BF, kind="ExternalInput")
    wdwm = nc.dram_tensor("wdwm", [128, 8, 9, 128], BF, kind="ExternalInput")
    wmlp2 = nc.dram_tensor("wmlp2", [128, 8, 256], BF, kind="ExternalInput")
    bprj = nc.dram_tensor("bprj", [128, 2], F32, kind="ExternalInput")
    b1 = nc.dram_tensor("b1", [128, 8], F32, kind="ExternalInput")
    bdw = nc.dram_tensor("bdw", [128, 8], F32, kind="ExternalInput")
    b2 = nc.dram_tensor("b2", [128, 2], F32, kind="ExternalInput")
    zmask = nc.dram_tensor("zmask", [128, 2, TN], BF, kind="ExternalInput")
    OUT = nc.dram_tensor("OUT", [128, 2, S1_T * TN], F32, kind="ExternalOutput")

    with TileContext(nc) as tc:
        with (
            tc.tile_pool(name="cst", bufs=1) as cst,
            tc.tile_pool(name="lop", bufs=5) as lop,
            tc.tile_pool(name="sb2", bufs=3) as sb2,
            tc.tile_pool(name="qp", bufs=4) as qp,
            tc.tile_pool(name="zp", bufs=4) as zp,
            tc.tile_pool(name="outp", bufs=5) as outp,
            tc.tile_pool(name="ylp", bufs=3) as ylp,
            tc.tile_pool(name="ps", bufs=2, space="PSUM") as ps,
        ):
            ones_t = cst.tile([128, 128], BF, tag="ones"); nc.sync.dma_start(ones_t[:], ones[:])
            kbd_t = cst.tile([128, 2, 152], BF, tag="kbd"); nc.sync.dma_start(kbd_t[:], kbd[:])
            pvbd_t = cst.tile([128, 2, 256], BF, tag="pvbd"); nc.sync.dma_start(pvbd_t[:], pvbd[:])
            obd_t = cst.tile([128, 4], BF, tag="obd"); nc.sync.dma_start(obd_t[:], onesbd[:])
            expd_t = cst.tile([4, 76], F32, tag="expd"); nc.sync.dma_start(expd_t[:], expd[:])
            bexp_t = cst.tile([128, 2], F32, tag="bexp"); nc.sync.dma_start(bexp_t[:], bexp[:])
            wqdw_t = cst.tile([128, 2, 9, 128], BF, tag="wqdw"); nc.sync.dma_start(wqdw_t[:], wqdw[:])
            wmlp1_t = cst.tile([128, 2, 1024], BF, tag="wmlp1"); nc.sync.dma_start(wmlp1_t[:], wmlp1[:])
            wdwm_t = cst.tile([128, 8, 9, 128], BF, tag="wdwm"); nc.sync.dma_start(wdwm_t[:], wdwm[:])
            wmlp2_t = cst.tile([128, 8, 256], BF, tag="wmlp2"); nc.sync.dma_start(wmlp2_t[:], wmlp2[:])
            bprj_t = cst.tile([128, 2], F32, tag="bprj"); nc.sync.dma_start(bprj_t[:], bprj[:])
            b1_t = cst.tile([128, 8], F32, tag="b1"); nc.sync.dma_start(b1_t[:], b1[:])
            bdw_t = cst.tile([128, 8], F32, tag="bdw"); nc.sync.dma_start(bdw_t[:], bdw[:])
            b2_t = cst.tile([128, 2], F32, tag="b2"); nc.sync.dma_start(b2_t[:], b2[:])
            zm_t = cst.tile([128, 2, TN], BF, tag="zm"); nc.sync.dma_start(zm_t[:], zmask[:])
            epsb = cst.tile([128, 1], F32, tag="epsb")
            nc.vector.memset(epsb[:], EPS)

            qtiles = {}
            ztiles = {}
            lo_a = {}
            out_a = {}
            yl_a = {}
            e_a = {}
            rz_a = {}
            qsb_a = {}
            f01_a = {}
            qd_a = {}
            en_a = {}
            sq_a = {}

            # per-tile LN stats -> (rstd, mu*rstd) bf16 full-width tiles
            def ln_tile(x0, x1, nm, sq=None):
                if sq is None:
                    sq = sb2.tile([128, 2, TN], BF, tag="sq", name="sq" + nm, bufs=3)
                    nc.gpsimd.tensor_tensor(sq[:, 0, :], x0, x0, op=AL.mult)
                    nc.gpsimd.tensor_tensor(sq[:, 1, :], x1, x1, op=AL.mult)
                s1 = ps.tile([128, TN], F32, tag="st", name="s1" + nm)
                nc.tensor.matmul(s1[:], ones_t[:], x0, start=True, stop=False)
                nc.tensor.matmul(s1[:], ones_t[:], x1, start=False, stop=True)
                s2 = ps.tile([128, TN], F32, tag="st", name="s2" + nm)
                nc.tensor.matmul(s2[:], ones_t[:], sq[:, 0, :], start=True, stop=False)
                nc.tensor.matmul(s2[:], ones_t[:], sq[:, 1, :], start=False, stop=True)
                mu2 = sb2.tile([128, TN], BF, tag="mu2", name="mu2" + nm)
                nc.scalar.activation(mu2[:], s1[:], AF.Square, scale=1.0 / C)
                mu_b = sb2.tile([128, TN], BF, tag="mu_b", name="mu_b" + nm)
                nc.scalar.activation(mu_b[:], s1[:], AF.Identity, scale=1.0 / C)
                var = sb2.tile([128, TN], F32, tag="var", name="var" + nm)
                nc.vector.scalar_tensor_tensor(var[:], s2[:], 1.0 / C, mu2[:],
                                               op0=AL.mult, op1=AL.subtract)
                nc.scalar.activation(var[:], var[:], AF.Ln, bias=epsb[:])
                rl = sb2.tile([128, TN], BF, tag="rl", name="rl" + nm)
                nc.scalar.activation(rl[:], var[:], AF.Exp, scale=-0.5)
                m2 = sb2.tile([128, TN], BF, tag="m2", name="m2" + nm)
                nc.vector.tensor_tensor(m2[:], mu_b[:], rl[:], op=AL.mult)
                return rl, m2

            def bq_dma(t):
                sl = slice(t * TN, (t + 1) * TN)
                lo_t = lop.tile([128, 2, TN], BF, tag="lo", name="lo_t")
                nc.sync.dma_start(lo_t[:], lo16[:, :, sl])
                lo_a[t] = lo_t

            def bq_sq(t):
                lo_t = lo_a[t]
                sq = sb2.tile([128, 2, TN], BF, tag="sq", name="sqq", bufs=3)
                nc.gpsimd.tensor_tensor(sq[:, 0, :], lo_t[:, 0, :], lo_t[:, 0, :], op=AL.mult)
                nc.gpsimd.tensor_tensor(sq[:, 1, :], lo_t[:, 1, :], lo_t[:, 1, :], op=AL.mult)
                sq_a[t] = sq

            def build_query(t):
                lo_t = lo_a[t]
                rl, m2 = ln_tile(lo_t[:, 0, :], lo_t[:, 1, :], "q", sq=sq_a.pop(t))
                qt = qp.tile([128, 2, 2 * QH + TN], BF, tag="qt")
                qtiles[t] = qt
                for ct in range(2):
                    nc.gpsimd.tensor_tensor(qt[:, ct, QH:QH + TN], lo_t[:, ct, :], rl[:],
                                            op=AL.mult)
                for ct in range(2):
                    nc.vector.tensor_tensor(qt[:, ct, QH:QH + TN], qt[:, ct, QH:QH + TN],
                                            m2[:], op=AL.subtract)
                if t == 0:
                    nc.vector.memset(qt[:, :, 0:QH], 0.0)
                else:
                    for ct in range(2):
                        nc.vector.tensor_copy(qt[:, ct, 0:QH],
                                              qtiles[t - 1][:, ct, TN:TN + QH])
                        nc.vector.tensor_copy(qtiles[t - 1][:, ct, QH + TN:],
                                              qt[:, ct, QH:2 * QH])
                if t == S2_T - 1:
                    nc.vector.memset(qt[:, :, QH + TN:], 0.0)

            def dw9(psum, wtile, src):
                for ti, (dr, dc) in enumerate(TAP_ORDER):
                    tap = (dr + 1) * 3 + (dc + 1)
                    off0 = QH + dr * 128
                    lhs = wtile[:, tap, :]
                    if dc == 0:
                        nc.tensor.matmul(psum[:], lhs, src[:, off0:off0 + TN],
                                         start=(ti == 0), stop=(ti == 8))
                    else:
                        rhs3 = src[:, off0:off0 + TN].rearrange("p (r w) -> p r w", w=128)
                        out3 = psum[:].rearrange("p (r w) -> p r w", w=128)
                        if dc == -1:
                            nc.tensor.matmul(out3[:, :, 1:128], lhs, rhs3[:, :, 0:127],
                                             start=False, stop=(ti == 8))
                        else:
                            nc.tensor.matmul(out3[:, :, 0:127], lhs, rhs3[:, :, 1:128],
                                             start=False, stop=(ti == 8))

            def attn_qdw(s, ct):
                qt = qtiles[s]
                if ct == 0:
                    qd = sb2.tile([128, 2, TN], BF, tag="qd")
                    qd_a[s] = qd
                qd = qd_a[s]
                qdp = ps.tile([128, TN], F32, tag="mm", name="qdp", bufs=3)
                dw9(qdp, wqdw_t[:, ct], qt[:, ct, :])
                nc.vector.tensor_copy(qd[:, ct, :], qdp[:])

            def attn_qk2(s, hf):
                qd = qd_a[s]
                if hf == 0:
                    e_a[s] = []
                    rz_a[s] = sb2.tile([4, 2, TN], F32, tag="rz", bufs=2, name="rz")
                lp = ps.tile([128, TN], F32, tag="at", name="lp", bufs=1)
                for kt in range(2):
                    nc.tensor.matmul(lp[0:76, :], kbd_t[:, kt, hf * 76:hf * 76 + 76],
                                     qd[:, kt, :], start=(kt == 0), stop=(kt == 1))
                e_h = sb2.tile([76, TN], BF, tag="eh%d" % hf)
                nc.scalar.activation(e_h[:], lp[0:76, :], AF.Exp, scale=-SCALE,
                                     bias=bexp_t[0:76, hf:hf + 1])
                e_a[s].append(e_h)
                zp_ = ps.tile([4, TN], F32, tag="at", name="zp_", bufs=1)
                nc.tensor.matmul(zp_[:], obd_t[0:76, :], e_h[:], start=True, stop=True)
                nc.vector.reciprocal_approx_fast(rz_a[s][:, hf, :], zp_[:])
                if hf == 1:
                    del qd_a[s]

            def attn_en(s, hf):
                e_ab = e_a[s]
                rz = rz_a[s]
                rzx = ps.tile([128, TN], F32, tag="at", name="rzx", bufs=1)
                nc.tensor.matmul(rzx[0:76, :], expd_t[:], rz[:, hf, :],
                                 start=True, stop=True)
                en = sb2.tile([76, TN], BF, tag="en%d" % hf, name="en")
                nc.vector.tensor_tensor(en[:], e_ab[hf][:], rzx[0:76, :], op=AL.mult)
                en_a.setdefault(s, []).append(en)
                if hf == 1:
                    del e_a[s]
                    del rz_a[s]

            def attn_proj(s, mt):
                en = en_a[s]
                if mt == 0:
                    out_t = outp.tile([128, 2, TN], BF, tag="out")
                    out_a[s] = out_t
                out_t = out_a[s]
                op_ = ps.tile([128, TN], F32, tag="mm", name="op_", bufs=3)
                for hf in range(2):
                    nc.tensor.matmul(op_[:], pvbd_t[0:76, hf, mt * 128:(mt + 1) * 128],
                                     en[hf][:], start=(hf == 0), stop=(hf == 1))
                nc.vector.scalar_tensor_tensor(out_t[:, mt, :], op_[:],
                                               bprj_t[:, mt:mt + 1],
                                               lo_a[s][:, mt, :], op0=AL.add, op1=AL.add)
                if mt == 1:
                    del en_a[s]
                    del lo_a[s]

            def attn_ln(s):
                out_t = out_a[s]
                ro, m2o = ln_tile(out_t[:, 0, :], out_t[:, 1, :], "o")
                yl_t = ylp.tile([128, 2, TN], BF, tag="yl")
                yl_a[s] = yl_t
                for ct in range(2):
                    nc.gpsimd.tensor_tensor(yl_t[:, ct, :], out_t[:, ct, :], ro[:],
                                            op=AL.mult)
                for ct in range(2):
                    nc.vector.tensor_tensor(yl_t[:, ct, :], yl_t[:, ct, :], m2o[:],
                                            op=AL.subtract)

            def build_z_pre(t):
                zt = zp.tile([128, 8, 2 * QH + TN], BF, tag="zt")
                ztiles[t] = zt

            def build_z_g(t, g):
                yl_t = yl_a[t]
                zt = ztiles[t]
                m1p = ps.tile([128, TN], F32, tag="mm", name="m1p", bufs=3)
                for kt in range(2):
                    nc.tensor.matmul(m1p[:], wmlp1_t[:, kt, g * 128:(g + 1) * 128],
                                     yl_t[:, kt, :], start=(kt == 0), stop=(kt == 1))
                if g % 2 == 0:
                    nc.scalar.activation(zt[:, g, QH:QH + TN], m1p[:], AF.Identity,
                                         bias=b1_t[:, g:g + 1])
                else:
                    nc.vector.tensor_scalar(zt[:, g, QH:QH + TN], m1p[:],
                                            b1_t[:, g:g + 1], None, op0=AL.add)
                if t == 0:
                    nc.vector.tensor_tensor(zt[:, g, QH:QH + TN], zt[:, g, QH:QH + TN],
                                            zm_t[:, 0, :], op=AL.mult)
                elif t == S2_T - 1:
                    nc.vector.tensor_tensor(zt[:, g, QH:QH + TN], zt[:, g, QH:QH + TN],
                                            zm_t[:, 1, :], op=AL.mult)

            def build_z_post(t):
                zt = ztiles[t]
                del yl_a[t]
                if t == 0:
                    nc.vector.memset(zt[:, :, 0:QH], 0.0)
                else:
                    for g in range(8):
                        nc.vector.tensor_copy(zt[:, g, 0:QH],
                                              ztiles[t - 1][:, g, TN:TN + QH])
                        nc.vector.tensor_copy(ztiles[t - 1][:, g, QH + TN:],
                                              zt[:, g, QH:2 * QH])
                if t == S2_T - 1:
                    nc.vector.memset(zt[:, :, QH + TN:], 0.0)

            def ffn_g(s, g):
                zt = ztiles[s]
                if g == 0:
                    f0 = ps.tile([128, TN], F32, tag="f01", name="f0")
                    f1 = ps.tile([128, TN], F32, tag="f01", name="f1")
                    f01_a[s] = (f0, f1)
                f0, f1 = f01_a[s]
                dwp = ps.tile([128, TN], F32, tag="mm", name="dwp", bufs=3)
                dw9(dwp, wdwm_t[:, g], zt[:, g, :])
                gel = sb2.tile([128, TN], BF, tag="gel")
                nc.scalar.activation(gel[:], dwp[:], AF.Gelu, bias=bdw_t[:, g:g + 1])
                nc.tensor.matmul(f0[:], wmlp2_t[:, g, 0:128], gel[:],
                                 start=(g == 0), stop=(g == 7))
                nc.tensor.matmul(f1[:], wmlp2_t[:, g, 128:256], gel[:],
                                 start=(g == 0), stop=(g == 7))

            def ffn_fin(s):
                f0, f1 = f01_a.pop(s)
                if s == 0:
                    px0, px1, o0 = 256, TN, 0
                elif s == S2_T - 1:
                    px0, px1, o0 = 0, 256, (S2_T - 1) * TN - 256
                else:
                    px0, px1, o0 = 0, TN, s * TN - 256
                n = px1 - px0
                for ct, fps in enumerate((f0, f1)):
                    fin = sb2.tile([128, TN], F32, tag="fin", name="fin")
                    nc.vector.scalar_tensor_tensor(fin[:, 0:n], fps[:, px0:px1],
                                                   b2_t[:, ct:ct + 1],
                                                   out_a[s][:, ct, px0:px1],
                                                   op0=AL.add, op1=AL.add)
                    nc.sync.dma_start(OUT[:, ct, o0:o0 + n], fin[:, 0:n])
                del out_a[s]

            # software pipeline with fine-grained PE-stream interleave:
            # z(t-3) mlp1 groups zip with the query-conv of attn(t-2); the
            # ffn(t-4) dw groups zip with the attention softmax/AV/proj chain
            # so the in-order PE queue never waits on an ACT/DVE consumer.
            for t in range(S2_T + 4):
                bz = 3 <= t < S2_T + 3      # build_z(t-3)
                qk = 2 <= t < S2_T + 2      # attn(t-2)
                fn = 4 <= t < S2_T + 4      # ffn(t-4)
                if bz:
                    build_z_pre(t - 3)
                    build_z_g(t - 3, 0)
                    build_z_g(t - 3, 1)
                if qk:
                    attn_qdw(t - 2, 0)
                if bz:
                    build_z_g(t - 3, 2)
                    build_z_g(t - 3, 3)
                if qk:
                    attn_qdw(t - 2, 1)
                if bz:
                    build_z_g(t - 3, 4)
                    build_z_g(t - 3, 5)
                if qk:
                    attn_qpw(t - 2)
                if bz:
                    build_z_g(t - 3, 6)
                    build_z_g(t - 3, 7)
                    build_z_post(t - 3)
                if qk:
                    attn_qk2(t - 2, 0)
                if fn:
                    ffn_g(t - 4, 0)
                if qk:
                    attn_qk2(t - 2, 1)
                if fn:
                    ffn_g(t - 4, 1)
                if qk:
                    attn_av1(t - 2, 0)
                if fn:
                    ffn_g(t - 4, 2)
                if qk:
                    attn_av1(t - 2, 1)
                if fn:
                    ffn_g(t - 4, 3)
                if qk:
                    attn_proj(t - 2, 0)
                if fn:
                    ffn_g(t - 4, 4)
                if qk:
                    attn_proj(t - 2, 1)
                if fn:
                    ffn_g(t - 4, 5)
                if qk:
                    attn_ln(t - 2)
                if fn:
                    ffn_g(t - 4, 6)
                    ffn_g(t - 4, 7)
                    ffn_fin(t - 4)
                    del ztiles[t - 4]
                if t < S2_T:
                    build_query(t)
                if qk:
                    del qtiles[t - 2]

    nc.finalize()
    return nc


# ----------------------------------------------------------------------------
# stage 2 builder
# ----------------------------------------------------------------------------

def build_stage2():
    """Fully fused stage 2: one software-pipelined loop per tile.

    iter t: build_z(t-3) | attn(t-2) | ffn(t-4) | build_query(t), with the
    ffn dw-conv groups interleaved into the attention chain so the in-order
    PE queue always has independent matmul work. Per-tile LN stats via
    ones-matmul broadcast + Ln/Exp (combined act table). The query 1x1 conv
    is folded into K (kbd = kq) and the output projection into V (pvbd), so
    q/av intermediates never materialize.
    """
    nc = bacc.Bacc()
    NPX = S2_T * TN
    lo16 = nc.dram_tensor("lo16", [128, 2, NPX], BF, kind="ExternalInput")
    ones = nc.dram_tensor("ones", [128, 128], BF, kind="ExternalInput")
    kbd = nc.dram_tensor("kbd", [128, 2, 152], BF, kind="ExternalInput")
    pvbd = nc.dram_tensor("pvbd", [128, 2, 256], BF, kind="ExternalInput")
    onesbd = nc.dram_tensor("onesbd", [128, 4], BF, kind="ExternalInput")
    expd = nc.dram_tensor("expd", [4, 76], F32, kind="ExternalInput")
    bexp = nc.dram_tensor("bexp", [128, 2], F32, kind="ExternalInput")
    wqdw = nc.dram_tensor("wqdw", [128, 2, 9, 128], BF, kind="ExternalInput")
    wmlp1 = nc.dram_tensor("wmlp1", [128, 2, 1024], BF, kind="ExternalInput")
    wdwm = nc.dram_tensor("wdwm", [128, 8, 9, 128], BF, kind="ExternalInput")
    wmlp2 = nc.dram_tensor("wmlp2", [128, 8, 256], BF, kind="ExternalInput")
    bprj = nc.dram_tensor("bprj", [128, 2], F32, kind="ExternalInput")
    b1 = nc.dram_tensor("b1", [128, 8], F32, kind="ExternalInput")
    bdw = nc.dram_tensor("bdw", [128, 8], F32, kind="ExternalInput")
    b2 = nc.dram_tensor("b2", [128, 2], F32, kind="ExternalInput")
    zmask = nc.dram_tensor("zmask", [128, 2, TN], BF, kind="ExternalInput")
    OUT = nc.dram_tensor("OUT", [128, 2, S1_T * TN], F32, kind="ExternalOutput")

    with TileContext(nc) as tc:
        with (
            tc.tile_pool(name="cst", bufs=1) as cst,
            tc.tile_pool(name="lop", bufs=5) as lop,
            tc.tile_pool(name="sb2", bufs=3) as sb2,
            tc.tile_pool(name="qp", bufs=4) as qp,
            tc.tile_pool(name="zp", bufs=4) as zp,
            tc.tile_pool(name="outp", bufs=5) as outp,
            tc.tile_pool(name="ylp", bufs=3) as ylp,
            tc.tile_pool(name="ps", bufs=2, space="PSUM") as ps,
        ):
            ones_t = cst.tile([128, 128], BF, tag="ones"); nc.sync.dma_start(ones_t[:], ones[:])
            kbd_t = cst.tile([128, 2, 152], BF, tag="kbd"); nc.sync.dma_start(kbd_t[:], kbd[:])
            pvbd_t = cst.tile([128, 2, 256], BF, tag="pvbd"); nc.sync.dma_start(pvbd_t[:], pvbd[:])
            obd_t = cst.tile([128, 4], BF, tag="obd"); nc.sync.dma_start(obd_t[:], onesbd[:])
            expd_t = cst.tile([4, 76], F32, tag="expd"); nc.sync.dma_start(expd_t[:], expd[:])
            bexp_t = cst.tile([128, 2], F32, tag="bexp"); nc.sync.dma_start(bexp_t[:], bexp[:])
            wqdw_t = cst.tile([128, 2, 9, 128], BF, tag="wqdw"); nc.sync.dma_start(wqdw_t[:], wqdw[:])
            wmlp1_t = cst.tile([128, 2, 1024], BF, tag="wmlp1"); nc.sync.dma_start(wmlp1_t[:], wmlp1[:])
            wdwm_t = cst.tile([128, 8, 9, 128], BF, tag="wdwm"); nc.sync.dma_start(wdwm_t[:], wdwm[:])
            wmlp2_t = cst.tile([128, 8, 256], BF, tag="wmlp2"); nc.sync.dma_start(wmlp2_t[:], wmlp2[:])
            bprj_t = cst.tile([128, 2], F32, tag="bprj"); nc.sync.dma_start(bprj_t[:], bprj[:])
            b1_t = cst.tile([128, 8], F32, tag="b1"); nc.sync.dma_start(b1_t[:], b1[:])
            bdw_t = cst.tile([128, 8], F32, tag="bdw"); nc.sync.dma_start(bdw_t[:], bdw[:])
            b2_t = cst.tile([128, 2], F32, tag="b2"); nc.sync.dma_start(b2_t[:], b2[:])
            zm_t = cst.tile([128, 2, TN], BF, tag="zm"); nc.sync.dma_start(zm_t[:], zmask[:])
            epsb = cst.tile([128, 1], F32, tag="epsb")
            nc.vector.memset(epsb[:], EPS)

            qtiles = {}
            ztiles = {}
            lo_a = {}
            out_a = {}
            yl_a = {}
            e_a = {}
            rz_a = {}
            f01_a = {}
            qd_a = {}
            en_a = {}
            sq_a = {}

            # per-tile LN stats -> (rstd, mu*rstd) bf16 full-width tiles
            def ln_tile(x0, x1, nm, sq=None):
                if sq is None:
                    sq = sb2.tile([128, 2, TN], BF, tag="sq", name="sq" + nm, bufs=3)
                    nc.gpsimd.tensor_tensor(sq[:, 0, :], x0, x0, op=AL.mult)
                    nc.gpsimd.tensor_tensor(sq[:, 1, :], x1, x1, op=AL.mult)
                s1 = ps.tile([128, TN], F32, tag="st", name="s1" + nm)
                nc.tensor.matmul(s1[:], ones_t[:], x0, start=True, stop=False)
                nc.tensor.matmul(s1[:], ones_t[:], x1, start=False, stop=True)
                s2 = ps.tile([128, TN], F32, tag="st", name="s2" + nm)
                nc.tensor.matmul(s2[:], ones_t[:], sq[:, 0, :], start=True, stop=False)
                nc.tensor.matmul(s2[:], ones_t[:], sq[:, 1, :], start=False, stop=True)
                mu2 = sb2.tile([128, TN], BF, tag="mu2", name="mu2" + nm)
                nc.scalar.activation(mu2[:], s1[:], AF.Square, scale=1.0 / C)
                mu_b = sb2.tile([128, TN], BF, tag="mu_b", name="mu_b" + nm)
                nc.scalar.activation(mu_b[:], s1[:], AF.Identity, scale=1.0 / C)
                var = sb2.tile([128, TN], F32, tag="var", name="var" + nm)
                nc.vector.scalar_tensor_tensor(var[:], s2[:], 1.0 / C, mu2[:],
                                               op0=AL.mult, op1=AL.subtract)
                nc.scalar.activation(var[:], var[:], AF.Ln, bias=epsb[:])
                rl = sb2.tile([128, TN], BF, tag="rl", name="rl" + nm)
                nc.scalar.activation(rl[:], var[:], AF.Exp, scale=-0.5)
                m2 = sb2.tile([128, TN], BF, tag="m2", name="m2" + nm)
                nc.vector.tensor_tensor(m2[:], mu_b[:], rl[:], op=AL.mult)
                return rl, m2

            def bq_dma(t):
                sl = slice(t * TN, (t + 1) * TN)
                lo_t = lop.tile([128, 2, TN], BF, tag="lo", name="lo_t")
                nc.sync.dma_start(lo_t[:], lo16[:, :, sl])
                lo_a[t] = lo_t

            def bq_sq(t):
                lo_t = lo_a[t]
                sq = sb2.tile([128, 2, TN], BF, tag="sq", name="sqq", bufs=3)
                nc.gpsimd.tensor_tensor(sq[:, 0, :], lo_t[:, 0, :], lo_t[:, 0, :], op=AL.mult)
                nc.gpsimd.tensor_tensor(sq[:, 1, :], lo_t[:, 1, :], lo_t[:, 1, :], op=AL.mult)
                sq_a[t] = sq

            def build_query(t):
                lo_t = lo_a[t]
                rl, m2 = ln_tile(lo_t[:, 0, :], lo_t[:, 1, :], "q", sq=sq_a.pop(t))
                qt = qp.tile([128, 2, 2 * QH + TN], BF, tag="qt")
                qtiles[t] = qt
                for ct in range(2):
                    nc.gpsimd.tensor_tensor(qt[:, ct, QH:QH + TN], lo_t[:, ct, :], rl[:],
                                            op=AL.mult)
                for ct in range(2):
                    nc.vector.tensor_tensor(qt[:, ct, QH:QH + TN], qt[:, ct, QH:QH + TN],
                                            m2[:], op=AL.subtract)
                if t == 0:
                    nc.vector.memset(qt[:, :, 0:QH], 0.0)
                else:
                    for ct in range(2):
                        nc.vector.tensor_copy(qt[:, ct, 0:QH],
                                              qtiles[t - 1][:, ct, TN:TN + QH])
                        nc.vector.tensor_copy(qtiles[t - 1][:, ct, QH + TN:],
                                              qt[:, ct, QH:2 * QH])
                if t == S2_T - 1:
                    nc.vector.memset(qt[:, :, QH + TN:], 0.0)

            def dw9(psum, wtile, src):
                for ti, (dr, dc) in enumerate(TAP_ORDER):
                    tap = (dr + 1) * 3 + (dc + 1)
                    off0 = QH + dr * 128
                    lhs = wtile[:, tap, :]
                    if dc == 0:
                        nc.tensor.matmul(psum[:], lhs, src[:, off0:off0 + TN],
                                         start=(ti == 0), stop=(ti == 8))
                    else:
                        rhs3 = src[:, off0:off0 + TN].rearrange("p (r w) -> p r w", w=128)
                        out3 = psum[:].rearrange("p (r w) -> p r w", w=128)
                        if dc == -1:
                            nc.tensor.matmul(out3[:, :, 1:128], lhs, rhs3[:, :, 0:127],
                                             start=False, stop=(ti == 8))
                        else:
                            nc.tensor.matmul(out3[:, :, 0:127], lhs, rhs3[:, :, 1:128],
                                             start=False, stop=(ti == 8))

            def attn_qdw(s, ct):
                qt = qtiles[s]
                if ct == 0:
                    qd = sb2.tile([128, 2, TN], BF, tag="qd")
                    qd_a[s] = qd
                qd = qd_a[s]
                qdp = ps.tile([128, TN], F32, tag="mm", name="qdp", bufs=3)
                dw9(qdp, wqdw_t[:, ct], qt[:, ct, :])
                nc.vector.tensor_copy(qd[:, ct, :], qdp[:])

            def attn_qk2(s, hf):
                qd = qd_a[s]
                if hf == 0:
                    e_a[s] = []
                    rz_a[s] = sb2.tile([4, 2, TN], F32, tag="rz", bufs=2, name="rz")
                lp = ps.tile([128, TN], F32, tag="at", name="lp", bufs=1)
                for kt in range(2):
                    nc.tensor.matmul(lp[0:76, :], kbd_t[:, kt, hf * 76:hf * 76 + 76],
                                     qd[:, kt, :], start=(kt == 0), stop=(kt == 1))
                e_h = sb2.tile([76, TN], BF, tag="eh%d" % hf)
                nc.scalar.activation(e_h[:], lp[0:76, :], AF.Exp, scale=-SCALE,
                                     bias=bexp_t[0:76, hf:hf + 1])
                e_a[s].append(e_h)
                zp_ = ps.tile([4, TN], F32, tag="at", name="zp_", bufs=1)
                nc.tensor.matmul(zp_[:], obd_t[0:76, :], e_h[:], start=True, stop=True)
                nc.vector.reciprocal_approx_fast(rz_a[s][:, hf, :], zp_[:])
                if hf == 1:
                    del qd_a[s]

            def attn_en(s, hf):
                e_ab = e_a[s]
                rz = rz_a[s]
                rzx = ps.tile([128, TN], F32, tag="at", name="rzx", bufs=1)
                nc.tensor.matmul(rzx[0:76, :], expd_t[:], rz[:, hf, :],
                                 start=True, stop=True)
                en = sb2.tile([76, TN], BF, tag="en%d" % hf, name="en")
                nc.vector.tensor_tensor(en[:], e_ab[hf][:], rzx[0:76, :], op=AL.mult)
                en_a.setdefault(s, []).append(en)
                if hf == 1:
                    del e_a[s]
                    del rz_a[s]

            def attn_proj(s, mt):
                en = en_a[s]
                if mt == 0:
                    out_t = outp.tile([128, 2, TN], BF, tag="out")
                    out_a[s] = out_t
                out_t = out_a[s]
                op_ = ps.tile([128, TN], F32, tag="mm", name="op_", bufs=3)
                for hf in range(2):
                    nc.tensor.matmul(op_[:], pvbd_t[0:76, hf, mt * 128:(mt + 1) * 128],
                                     en[hf][:], start=(hf == 0), stop=(hf == 1))
                nc.vector.scalar_tensor_tensor(out_t[:, mt, :], op_[:],
                                               bprj_t[:, mt:mt + 1],
                                               lo_a[s][:, mt, :], op0=AL.add, op1=AL.add)
                if mt == 1:
                    del en_a[s]
                    del lo_a[s]

            def attn_ln(s):
                out_t = out_a[s]
                ro, m2o = ln_tile(out_t[:, 0, :], out_t[:, 1, :], "o")
                yl_t = ylp.tile([128, 2, TN], BF, tag="yl")
                yl_a[s] = yl_t
                for ct in range(2):
                    nc.gpsimd.tensor_tensor(yl_t[:, ct, :], out_t[:, ct, :], ro[:],
                                            op=AL.mult)
                for ct in range(2):
                    nc.vector.tensor_tensor(yl_t[:, ct, :], yl_t[:, ct, :], m2o[:],
                                            op=AL.subtract)

            def build_z_pre(t):
                zt = zp.tile([128, 8, 2 * QH + TN], BF, tag="zt")
                ztiles[t] = zt

            def build_z_g(t, g):
                yl_t = yl_a[t]
                zt = ztiles[t]
                m1p = ps.tile([128, TN], F32, tag="mm", name="m1p", bufs=3)
                for kt in range(2):
                    nc.tensor.matmul(m1p[:], wmlp1_t[:, kt, g * 128:(g + 1) * 128],
                                     yl_t[:, kt, :], start=(kt == 0), stop=(kt == 1))
                if g % 2 == 0:
                    nc.scalar.activation(zt[:, g, QH:QH + TN], m1p[:], AF.Identity,
                                         bias=b1_t[:, g:g + 1])
                else:
                    nc.vector.tensor_scalar(zt[:, g, QH:QH + TN], m1p[:],
                                            b1_t[:, g:g + 1], None, op0=AL.add)
                if t == 0:
                    nc.vector.tensor_tensor(zt[:, g, QH:QH + TN], zt[:, g, QH:QH + TN],
                                            zm_t[:, 0, :], op=AL.mult)
                elif t == S2_T - 1:
                    nc.vector.tensor_tensor(zt[:, g, QH:QH + TN], zt[:, g, QH:QH + TN],
                                            zm_t[:, 1, :], op=AL.mult)

            def build_z_post(t):
                zt = ztiles[t]
                del yl_a[t]
                if t == 0:
                    nc.vector.memset(zt[:, :, 0:QH], 0.0)
                else:
                    for g in range(8):
                        nc.vector.tensor_copy(zt[:, g, 0:QH],
                                              ztiles[t - 1][:, g, TN:TN + QH])
                        nc.vector.tensor_copy(ztiles[t - 1][:, g, QH + TN:],
                                              zt[:, g, QH:2 * QH])
                if t == S2_T - 1:
                    nc.vector.memset(zt[:, :, QH + TN:], 0.0)

            def ffn_g(s, g):
                zt = ztiles[s]
                if g == 0:
                    f0 = ps.tile([128, TN], F32, tag="f01", name="f0")
                    f1 = ps.tile([128, TN], F32, tag="f01", name="f1")
                    f01_a[s] = (f0, f1)
                f0, f1 = f01_a[s]
                dwp = ps.tile([128, TN], F32, tag="mm", name="dwp", bufs=3)
                dw9(dwp, wdwm_t[:, g], zt[:, g, :])
                gel = sb2.tile([128, TN], BF, tag="gel")
                nc.scalar.activation(gel[:], dwp[:], AF.Gelu, bias=bdw_t[:, g:g + 1])
                nc.tensor.matmul(f0[:], wmlp2_t[:, g, 0:128], gel[:],
                                 start=(g == 0), stop=(g == 7))
                nc.tensor.matmul(f1[:], wmlp2_t[:, g, 128:256], gel[:],
                                 start=(g == 0), stop=(g == 7))

            def ffn_fin(s):
                f0, f1 = f01_a.pop(s)
                if s == 0:
                    px0, px1, o0 = 256, TN, 0
                elif s == S2_T - 1:
                    px0, px1, o0 = 0, 256, (S2_T - 1) * TN - 256
                else:
                    px0, px1, o0 = 0, TN, s * TN - 256
                n = px1 - px0
                for ct, fps in enumerate((f0, f1)):
                    fin = sb2.tile([128, TN], F32, tag="fin", name="fin")
                    nc.vector.scalar_tensor_tensor(fin[:, 0:n], fps[:, px0:px1],
                                                   b2_t[:, ct:ct + 1],
                                                   out_a[s][:, ct, px0:px1],
                                                   op0=AL.add, op1=AL.add)
                    nc.sync.dma_start(OUT[:, ct, o0:o0 + n], fin[:, 0:n])
                del out_a[s]

            # software pipeline: attn(t-2) | z(t-4) | ffn(t-5) | query(t).
            # lo DMA prefetched one iter ahead and its squares issued at iter
            # start, so the tail LN-stat matmuls never wait; build_z consumes
            # yl with a full iteration of slack; ffn dw groups fill the PE
            # while the attention and LN chains run on ACT/DVE.
            for t in range(S2_T + 5):
                bz = 4 <= t < S2_T + 4      # build_z(t-4)
                qk = 2 <= t < S2_T + 2      # attn(t-2)
                fn = 5 <= t < S2_T + 5      # ffn(t-5)
                if t == 0:
                    bq_dma(0)
                if t + 1 < S2_T:
                    bq_dma(t + 1)
                if t < S2_T:
                    bq_sq(t)
                if bz:
                    build_z_pre(t - 4)
                    for g in range(8):
                        build_z_g(t - 4, g)
                    build_z_post(t - 4)
                if qk:
                    attn_qdw(t - 2, 0)
                    attn_qdw(t - 2, 1)
                    # both attention Exps back-to-back; Gelu block follows =>
                    # two act-table switches per iteration total
                    attn_qk2(t - 2, 0)
                    attn_qk2(t - 2, 1)
                if fn:
                    for g in range(8):
                        ffn_g(t - 5, g)
                if qk:
                    attn_en(t - 2, 0)
                    attn_en(t - 2, 1)
                    attn_proj(t - 2, 0)
                    attn_proj(t - 2, 1)
                if fn:
                    ffn_fin(t - 5)
                    del ztiles[t - 5]
                if qk:
                    attn_ln(t - 2)
                if t < S2_T:
                    build_query(t)
                if qk:
                    del qtiles[t - 2]

    nc.finalize()
    return nc


# ----------------------------------------------------------------------------
# stage 2 builder
# ----------------------------------------------------------------------------

def build_stage2():
    """Fully fused stage 2: one software-pipelined loop per tile.

    iter t: build_query(t) | attn(t-1) | build_z(t-2) | ffn(t-3).
    Per-tile LN stats (ones-matmul broadcast + Ln/Exp from the combined act
    table); out/yl kept as rotating bf16 SBUF tiles; FFN matmuls interleave
    with the attention chain so TensorE never idles past the HAM window.
    """
    nc = bacc.Bacc()
    NPX = S2_T * TN
    lo16 = nc.dram_tensor("lo16", [128, 2, NPX], BF, kind="ExternalInput")
    ones = nc.dram_tensor("ones", [128, 128], BF, kind="ExternalInput")
    kbd = nc.dram_tensor("kbd", [128, 2, 152], BF, kind="ExternalInput")
    pvbd = nc.dram_tensor("pvbd", [128, 2, 256], BF, kind="ExternalInput")
    onesbd = nc.dram_tensor("onesbd", [128, 4], BF, kind="ExternalInput")
    expd = nc.dram_tensor("expd", [4, 76], F32, kind="ExternalInput")
    bexp = nc.dram_tensor("bexp", [128, 2], F32, kind="ExternalInput")
    wqdw = nc.dram_tensor("wqdw", [128, 2, 9, 128], BF, kind="ExternalInput")
    wmlp1 = nc.dram_tensor("wmlp1", [128, 2, 1024], BF, kind="ExternalInput")
    wdwm = nc.dram_tensor("wdwm", [128, 8, 9, 128], BF, kind="ExternalInput")
    wmlp2 = nc.dram_tensor("wmlp2", [128, 8, 256], BF, kind="ExternalInput")
    bprj = nc.dram_tensor("bprj", [128, 2], F32, kind="ExternalInput")
    b1 = nc.dram_tensor("b1", [128, 8], F32, kind="ExternalInput")
    bdw = nc.dram_tensor("bdw", [128, 8], F32, kind="ExternalInput")
    b2 = nc.dram_tensor("b2", [128, 2], F32, kind="ExternalInput")
    zmask = nc.dram_tensor("zmask", [128, 2, TN], BF, kind="ExternalInput")
    OUT = nc.dram_tensor("OUT", [128, 2, S1_T * TN], F32, kind="ExternalOutput")

    with TileContext(nc) as tc:
        with (
            tc.tile_pool(name="cst", bufs=1) as cst,
            tc.tile_pool(name="lop", bufs=5) as lop,
            tc.tile_pool(name="sb2", bufs=3) as sb2,
            tc.tile_pool(name="qp", bufs=4) as qp,
            tc.tile_pool(name="zp", bufs=4) as zp,
            tc.tile_pool(name="outp", bufs=5) as outp,
            tc.tile_pool(name="ylp", bufs=3) as ylp,
            tc.tile_pool(name="ps", bufs=2, space="PSUM") as ps,
        ):
            ones_t = cst.tile([128, 128], BF, tag="ones"); nc.sync.dma_start(ones_t[:], ones[:])
            kbd_t = cst.tile([128, 2, 152], BF, tag="kbd"); nc.sync.dma_start(kbd_t[:], kbd[:])
            pvbd_t = cst.tile([128, 2, 256], BF, tag="pvbd"); nc.sync.dma_start(pvbd_t[:], pvbd[:])
            obd_t = cst.tile([128, 4], BF, tag="obd"); nc.sync.dma_start(obd_t[:], onesbd[:])
            expd_t = cst.tile([4, 76], F32, tag="expd"); nc.sync.dma_start(expd_t[:], expd[:])
            bexp_t = cst.tile([128, 2], F32, tag="bexp"); nc.sync.dma_start(bexp_t[:], bexp[:])
            wqdw_t = cst.tile([128, 2, 9, 128], BF, tag="wqdw"); nc.sync.dma_start(wqdw_t[:], wqdw[:])
            wmlp1_t = cst.tile([128, 2, 1024], BF, tag="wmlp1"); nc.sync.dma_start(wmlp1_t[:], wmlp1[:])
            wdwm_t = cst.tile([128, 8, 9, 128], BF, tag="wdwm"); nc.sync.dma_start(wdwm_t[:], wdwm[:])
            wmlp2_t = cst.tile([128, 8, 256], BF, tag="wmlp2"); nc.sync.dma_start(wmlp2_t[:], wmlp2[:])
            bprj_t = cst.tile([128, 2], F32, tag="bprj"); nc.sync.dma_start(bprj_t[:], bprj[:])
            b1_t = cst.tile([128, 8], F32, tag="b1"); nc.sync.dma_start(b1_t[:], b1[:])
            bdw_t = cst.tile([128, 8], F32, tag="bdw"); nc.sync.dma_start(bdw_t[:], bdw[:])
            b2_t = cst.tile([128, 2], F32, tag="b2"); nc.sync.dma_start(b2_t[:], b2[:])
            zm_t = cst.tile([128, 2, TN], BF, tag="zm"); nc.sync.dma_start(zm_t[:], zmask[:])
            epsb = cst.tile([128, 1], F32, tag="epsb")
            nc.vector.memset(epsb[:], EPS)

            qtiles = {}
            ztiles = {}
            lo_a = {}
            out_a = {}
            yl_a = {}
            e_a = {}
            rz_a = {}
            qsb_a = {}
            f01_a = {}
            qd_a = {}
            en_a = {}
            sq_a = {}

            # per-tile LN stats -> (rstd, mu*rstd) bf16 full-width tiles
            def ln_tile(x0, x1, nm, sq=None):
                if sq is None:
                    sq = sb2.tile([128, 2, TN], BF, tag="sq", name="sq" + nm, bufs=3)
                    nc.gpsimd.tensor_tensor(sq[:, 0, :], x0, x0, op=AL.mult)
                    nc.gpsimd.tensor_tensor(sq[:, 1, :], x1, x1, op=AL.mult)
                s1 = ps.tile([128, TN], F32, tag="st", name="s1" + nm)
                nc.tensor.matmul(s1[:], ones_t[:], x0, start=True, stop=False)
                nc.tensor.matmul(s1[:], ones_t[:], x1, start=False, stop=True)
                s2 = ps.tile([128, TN], F32, tag="st", name="s2" + nm)
                nc.tensor.matmul(s2[:], ones_t[:], sq[:, 0, :], start=True, stop=False)
                nc.tensor.matmul(s2[:], ones_t[:], sq[:, 1, :], start=False, stop=True)
                mu2 = sb2.tile([128, TN], BF, tag="mu2", name="mu2" + nm)
                nc.scalar.activation(mu2[:], s1[:], AF.Square, scale=1.0 / C)
                mu_b = sb2.tile([128, TN], BF, tag="mu_b", name="mu_b" + nm)
                nc.scalar.activation(mu_b[:], s1[:], AF.Identity, scale=1.0 / C)
                var = sb2.tile([128, TN], F32, tag="var", name="var" + nm)
                nc.vector.scalar_tensor_tensor(var[:], s2[:], 1.0 / C, mu2[:],
                                               op0=AL.mult, op1=AL.subtract)
                nc.scalar.activation(var[:], var[:], AF.Ln, bias=epsb[:])
                rl = sb2.tile([128, TN], BF, tag="rl", name="rl" + nm)
                nc.scalar.activation(rl[:], var[:], AF.Exp, scale=-0.5)
                m2 = sb2.tile([128, TN], BF, tag="m2", name="m2" + nm)
                nc.vector.tensor_tensor(m2[:], mu_b[:], rl[:], op=AL.mult)
                return rl, m2

            def bq_dma(t):
                sl = slice(t * TN, (t + 1) * TN)
                lo_t = lop.tile([128, 2, TN], BF, tag="lo", name="lo_t")
                nc.sync.dma_start(lo_t[:], lo16[:, :, sl])
                lo_a[t] = lo_t

            def bq_sq(t):
                lo_t = lo_a[t]
                sq = sb2.tile([128, 2, TN], BF, tag="sq", name="sqq", bufs=3)
                nc.gpsimd.tensor_tensor(sq[:, 0, :], lo_t[:, 0, :], lo_t[:, 0, :], op=AL.mult)
                nc.gpsimd.tensor_tensor(sq[:, 1, :], lo_t[:, 1, :], lo_t[:, 1, :], op=AL.mult)
                sq_a[t] = sq

            def build_query(t):
                lo_t = lo_a[t]
                rl, m2 = ln_tile(lo_t[:, 0, :], lo_t[:, 1, :], "q", sq=sq_a.pop(t))
                qt = qp.tile([128, 2, 2 * QH + TN], BF, tag="qt")
                qtiles[t] = qt
                for ct in range(2):
                    nc.gpsimd.tensor_tensor(qt[:, ct, QH:QH + TN], lo_t[:, ct, :], rl[:],
                                            op=AL.mult)
                for ct in range(2):
                    nc.vector.tensor_tensor(qt[:, ct, QH:QH + TN], qt[:, ct, QH:QH + TN],
                                            m2[:], op=AL.subtract)
                if t == 0:
                    nc.vector.memset(qt[:, :, 0:QH], 0.0)
                else:
                    for ct in range(2):
                        nc.vector.tensor_copy(qt[:, ct, 0:QH],
                                              qtiles[t - 1][:, ct, TN:TN + QH])
                        nc.vector.tensor_copy(qtiles[t - 1][:, ct, QH + TN:],
                                              qt[:, ct, QH:2 * QH])
                if t == S2_T - 1:
                    nc.vector.memset(qt[:, :, QH + TN:], 0.0)

            def dw9(psum, wtile, src):
                for ti, (dr, dc) in enumerate(TAP_ORDER):
                    tap = (dr + 1) * 3 + (dc + 1)
                    off0 = QH + dr * 128
                    lhs = wtile[:, tap, :]
                    if dc == 0:
                        nc.tensor.matmul(psum[:], lhs, src[:, off0:off0 + TN],
                                         start=(ti == 0), stop=(ti == 8))
                    else:
                        rhs3 = src[:, off0:off0 + TN].rearrange("p (r w) -> p r w", w=128)
                        out3 = psum[:].rearrange("p (r w) -> p r w", w=128)
                        if dc == -1:
                            nc.tensor.matmul(out3[:, :, 1:128], lhs, rhs3[:, :, 0:127],
                                             start=False, stop=(ti == 8))
                        else:
                            nc.tensor.matmul(out3[:, :, 0:127], lhs, rhs3[:, :, 1:128],
                                             start=False, stop=(ti == 8))

            def attn_qdw(s, ct):
                qt = qtiles[s]
                if ct == 0:
                    qd = sb2.tile([128, 2, TN], BF, tag="qd")
                    qd_a[s] = qd
                qd = qd_a[s]
                qdp = ps.tile([128, TN], F32, tag="mm", name="qdp", bufs=3)
                dw9(qdp, wqdw_t[:, ct], qt[:, ct, :])
                nc.vector.tensor_copy(qd[:, ct, :], qdp[:])

            def attn_qk2(s, hf):
                qd = qd_a[s]
                if hf == 0:
                    e_a[s] = []
                    rz_a[s] = sb2.tile([4, 2, TN], F32, tag="rz", bufs=2, name="rz")
                lp = ps.tile([128, TN], F32, tag="at", name="lp", bufs=1)
                for kt in range(2):
                    nc.tensor.matmul(lp[0:76, :], kbd_t[:, kt, hf * 76:hf * 76 + 76],
                                     qd[:, kt, :], start=(kt == 0), stop=(kt == 1))
                e_h = sb2.tile([76, TN], BF, tag="eh%d" % hf)
                nc.scalar.activation(e_h[:], lp[0:76, :], AF.Exp, scale=-SCALE,
                                     bias=bexp_t[0:76, hf:hf + 1])
                e_a[s].append(e_h)
                zp_ = ps.tile([4, TN], F32, tag="at", name="zp_", bufs=1)
                nc.tensor.matmul(zp_[:], obd_t[0:76, :], e_h[:], start=True, stop=True)
                nc.vector.reciprocal_approx_fast(rz_a[s][:, hf, :], zp_[:])
                if hf == 1:
                    del qd_a[s]

            def attn_en(s, hf):
                e_ab = e_a[s]
                rz = rz_a[s]
                rzx = ps.tile([128, TN], F32, tag="at", name="rzx", bufs=1)
                nc.tensor.matmul(rzx[0:76, :], expd_t[:], rz[:, hf, :],
                                 start=True, stop=True)
                en = sb2.tile([76, TN], BF, tag="en%d" % hf, name="en")
                nc.vector.tensor_tensor(en[:], e_ab[hf][:], rzx[0:76, :], op=AL.mult)
                en_a.setdefault(s, []).append(en)
                if hf == 1:
                    del e_a[s]
                    del rz_a[s]

            def attn_proj(s, mt):
                en = en_a[s]
                if mt == 0:
                    out_t = outp.tile([128, 2, TN], BF, tag="out")
                    out_a[s] = out_t
                out_t = out_a[s]
                op_ = ps.tile([128, TN], F32, tag="mm", name="op_", bufs=3)
                for hf in range(2):
                    nc.tensor.matmul(op_[:], pvbd_t[0:76, hf, mt * 128:(mt + 1) * 128],
                                     en[hf][:], start=(hf == 0), stop=(hf == 1))
                nc.vector.scalar_tensor_tensor(out_t[:, mt, :], op_[:],
                                               bprj_t[:, mt:mt + 1],
                                               lo_a[s][:, mt, :], op0=AL.add, op1=AL.add)
                if mt == 1:
                    del en_a[s]
                    del lo_a[s]

            def attn_ln(s):
                out_t = out_a[s]
                ro, m2o = ln_tile(out_t[:, 0, :], out_t[:, 1, :], "o")
                yl_t = ylp.tile([128, 2, TN], BF, tag="yl")
                yl_a[s] = yl_t
                for ct in range(2):
                    nc.gpsimd.tensor_tensor(yl_t[:, ct, :], out_t[:, ct, :], ro[:],
                                            op=AL.mult)
                for ct in range(2):
                    nc.vector.tensor_tensor(yl_t[:, ct, :], yl_t[:, ct, :], m2o[:],
                                            op=AL.subtract)

            def build_z_pre(t):
                zt = zp.tile([128, 8, 2 * QH + TN], BF, tag="zt")
                ztiles[t] = zt

            def build_z_g(t, g):
                yl_t = yl_a[t]
                zt = ztiles[t]
                m1p = ps.tile([128, TN], F32, tag="mm", name="m1p", bufs=3)
                for kt in range(2):
                    nc.tensor.matmul(m1p[:], wmlp1_t[:, kt, g * 128:(g + 1) * 128],
                                     yl_t[:, kt, :], start=(kt == 0), stop=(kt == 1))
                if g % 2 == 0:
                    nc.scalar.activation(zt[:, g, QH:QH + TN], m1p[:], AF.Identity,
                                         bias=b1_t[:, g:g + 1])
                else:
                    nc.vector.tensor_scalar(zt[:, g, QH:QH + TN], m1p[:],
                                            b1_t[:, g:g + 1], None, op0=AL.add)
                if t == 0:
                    nc.vector.tensor_tensor(zt[:, g, QH:QH + TN], zt[:, g, QH:QH + TN],
                                            zm_t[:, 0, :], op=AL.mult)
                elif t == S2_T - 1:
                    nc.vector.tensor_tensor(zt[:, g, QH:QH + TN], zt[:, g, QH:QH + TN],
                                            zm_t[:, 1, :], op=AL.mult)

            def build_z_post(t):
                zt = ztiles[t]
                del yl_a[t]
                if t == 0:
                    nc.vector.memset(zt[:, :, 0:QH], 0.0)
                else:
                    for g in range(8):
                        nc.vector.tensor_copy(zt[:, g, 0:QH],
                                              ztiles[t - 1][:, g, TN:TN + QH])
                        nc.vector.tensor_copy(ztiles[t - 1][:, g, QH + TN:],
                                              zt[:, g, QH:2 * QH])
                if t == S2_T - 1:
                    nc.vector.memset(zt[:, :, QH + TN:], 0.0)

            def ffn_g(s, g):
                zt = ztiles[s]
                if g == 0:
                    f0 = ps.tile([128, TN], F32, tag="f01", name="f0")
                    f1 = ps.tile([128, TN], F32, tag="f01", name="f1")
                    f01_a[s] = (f0, f1)
                f0, f1 = f01_a[s]
                dwp = ps.tile([128, TN], F32, tag="mm", name="dwp", bufs=3)
                dw9(dwp, wdwm_t[:, g], zt[:, g, :])
                gel = sb2.tile([128, TN], BF, tag="gel")
                nc.scalar.activation(gel[:], dwp[:], AF.Gelu, bias=bdw_t[:, g:g + 1])
                nc.tensor.matmul(f0[:], wmlp2_t[:, g, 0:128], gel[:],
                                 start=(g == 0), stop=(g == 7))
                nc.tensor.matmul(f1[:], wmlp2_t[:, g, 128:256], gel[:],
                                 start=(g == 0), stop=(g == 7))

            def ffn_fin(s):
                f0, f1 = f01_a.pop(s)
                if s == 0:
                    px0, px1, o0 = 256, TN, 0
                elif s == S2_T - 1:
                    px0, px1, o0 = 0, 256, (S2_T - 1) * TN - 256
                else:
                    px0, px1, o0 = 0, TN, s * TN - 256
                n = px1 - px0
                for ct, fps in enumerate((f0, f1)):
                    fin = sb2.tile([128, TN], F32, tag="fin", name="fin")
                    nc.vector.scalar_tensor_tensor(fin[:, 0:n], fps[:, px0:px1],
                                                   b2_t[:, ct:ct + 1],
                                                   out_a[s][:, ct, px0:px1],
                                                   op0=AL.add, op1=AL.add)
                    nc.sync.dma_start(OUT[:, ct, o0:o0 + n], fin[:, 0:n])
                del out_a[s]

            # software pipeline: z(t-3) | attn(t-2) | ffn(t-4) | query(t),
            # coarse chunks so each engine queue drains in dependency order,
            # ffn dw groups filling the PE while attention chains run.
            for t in range(S2_T + 4):
                bz = 3 <= t < S2_T + 3      # build_z(t-3)
                qk = 2 <= t < S2_T + 2      # attn(t-2)
                fn = 4 <= t < S2_T + 4      # ffn(t-4)
                if bz:
                    build_z_pre(t - 3)
                    for g in range(8):
                        build_z_g(t - 3, g)
                    build_z_post(t - 3)
                if qk:
                    attn_qdw(t - 2, 0)
                    attn_qdw(t - 2, 1)
                    # both attention Exps back-to-back: exactly one act-table
                    # switch into Gelu below and one back per iteration
                    attn_qk2(t - 2, 0)
                    attn_qk2(t - 2, 1)
                if fn:
                    for g in range(8):
                        ffn_g(t - 4, g)
                if qk:
                    attn_en(t - 2, 0)
                    attn_en(t - 2, 1)
                    attn_proj(t - 2, 0)
                    attn_proj(t - 2, 1)
                if fn:
                    ffn_fin(t - 4)
                    del ztiles[t - 4]
                if qk:
                    attn_ln(t - 2)
                if t < S2_T:
                    build_query(t)
                if qk:
                    del qtiles[t - 2]

    nc.finalize()
    return nc


# ----------------------------------------------------------------------------
# stage 2 builder
# ----------------------------------------------------------------------------

def build_stage2():
    """Fully fused stage 2: one software-pipelined loop per tile.

    iter t: build_query(t) | attn(t-1) | build_z(t-2) | ffn(t-3).
    Per-tile LN stats (ones-matmul broadcast + Ln/Exp from the combined act
    table); out/yl kept as rotating bf16 SBUF tiles; FFN matmuls interleave
    with the attention chain so TensorE never idles past the HAM window.
    """
    nc = bacc.Bacc()
    NPX = S2_T * TN
    lo16 = nc.dram_tensor("lo16", [128, 2, NPX], BF, kind="ExternalInput")
    ones = nc.dram_tensor("ones", [128, 128], BF, kind="ExternalInput")
    kbd = nc.dram_tensor("kbd", [128, 2, 152], BF, kind="ExternalInput")
    pvbd = nc.dram_tensor("pvbd", [128, 2, 256], BF, kind="ExternalInput")
    onesbd = nc.dram_tensor("onesbd", [128, 4], BF, kind="ExternalInput")
    expd = nc.dram_tensor("expd", [4, 76], F32, kind="ExternalInput")
    bexp = nc.dram_tensor("bexp", [128, 2], F32, kind="ExternalInput")
    wqdw = nc.dram_tensor("wqdw", [128, 2, 9, 128], BF, kind="ExternalInput")
    wmlp1 = nc.dram_tensor("wmlp1", [128, 2, 1024], BF, kind="ExternalInput")
    wdwm = nc.dram_tensor("wdwm", [128, 8, 9, 128], BF, kind="ExternalInput")
    wmlp2 = nc.dram_tensor("wmlp2", [128, 8, 256], BF, kind="ExternalInput")
    bprj = nc.dram_tensor("bprj", [128, 2], F32, kind="ExternalInput")
    b1 = nc.dram_tensor("b1", [128, 8], F32, kind="ExternalInput")
    bdw = nc.dram_tensor("bdw", [128, 8], F32, kind="ExternalInput")
    b2 = nc.dram_tensor("b2", [128, 2], F32, kind="ExternalInput")
    zmask = nc.dram_tensor("zmask", [128, 2, TN], BF, kind="ExternalInput")
    OUT = nc.dram_tensor("OUT", [128, 2, S1_T * TN], F32, kind="ExternalOutput")

    with TileContext(nc) as tc:
        with (
            tc.tile_pool(name="cst", bufs=1) as cst,
            tc.tile_pool(name="lop", bufs=5) as lop,
            tc.tile_pool(name="sb2", bufs=3) as sb2,
            tc.tile_pool(name="qp", bufs=4) as qp,
            tc.tile_pool(name="zp", bufs=4) as zp,
            tc.tile_pool(name="outp", bufs=5) as outp,
            tc.tile_pool(name="ylp", bufs=3) as ylp,
            tc.tile_pool(name="ps", bufs=2, space="PSUM") as ps,
        ):
            ones_t = cst.tile([128, 128], BF, tag="ones"); nc.sync.dma_start(ones_t[:], ones[:])
            kbd_t = cst.tile([128, 2, 152], BF, tag="kbd"); nc.sync.dma_start(kbd_t[:], kbd[:])
            pvbd_t = cst.tile([128, 2, 256], BF, tag="pvbd"); nc.sync.dma_start(pvbd_t[:], pvbd[:])
            obd_t = cst.tile([128, 4], BF, tag="obd"); nc.sync.dma_start(obd_t[:], onesbd[:])
            expd_t = cst.tile([4, 76], F32, tag="expd"); nc.sync.dma_start(expd_t[:], expd[:])
            bexp_t = cst.tile([128, 2], F32, tag="bexp"); nc.sync.dma_start(bexp_t[:], bexp[:])
            wqdw_t = cst.tile([128, 2, 9, 128], BF, tag="wqdw"); nc.sync.dma_start(wqdw_t[:], wqdw[:])
            wmlp1_t = cst.tile([128, 2, 1024], BF, tag="wmlp1"); nc.sync.dma_start(wmlp1_t[:], wmlp1[:])
            wdwm_t = cst.tile([128, 8, 9, 128], BF, tag="wdwm"); nc.sync.dma_start(wdwm_t[:], wdwm[:])
            wmlp2_t = cst.tile([128, 8, 256], BF, tag="wmlp2"); nc.sync.dma_start(wmlp2_t[:], wmlp2[:])
            bprj_t = cst.tile([128, 2], F32, tag="bprj"); nc.sync.dma_start(bprj_t[:], bprj[:])
            b1_t = cst.tile([128, 8], F32, tag="b1"); nc.sync.dma_start(b1_t[:], b1[:])
            bdw_t = cst.tile([128, 8], F32, tag="bdw"); nc.sync.dma_start(bdw_t[:], bdw[:])
            b2_t = cst.tile([128, 2], F32, tag="b2"); nc.sync.dma_start(b2_t[:], b2[:])
            zm_t = cst.tile([128, 2, TN], BF, tag="zm"); nc.sync.dma_start(zm_t[:], zmask[:])
            epsb = cst.tile([128, 1], F32, tag="epsb")
            nc.vector.memset(epsb[:], EPS)

            qtiles = {}
            ztiles = {}
            lo_a = {}
            out_a = {}
            yl_a = {}
            e_a = {}
            rz_a = {}
            qsb_a = {}
            f01_a = {}
            qd_a = {}
            en_a = {}
            sq_a = {}

            # per-tile LN stats -> (rstd, mu*rstd) bf16 full-width tiles
            def ln_tile(x0, x1, nm, sq=None):
                if sq is None:
                    sq = sb2.tile([128, 2, TN], BF, tag="sq", name="sq" + nm, bufs=3)
                    nc.gpsimd.tensor_tensor(sq[:, 0, :], x0, x0, op=AL.mult)
                    nc.gpsimd.tensor_tensor(sq[:, 1, :], x1, x1, op=AL.mult)
                s1 = ps.tile([128, TN], F32, tag="st", name="s1" + nm)
                nc.tensor.matmul(s1[:], ones_t[:], x0, start=True, stop=False)
                nc.tensor.matmul(s1[:], ones_t[:], x1, start=False, stop=True)
                s2 = ps.tile([128, TN], F32, tag="st", name="s2" + nm)
                nc.tensor.matmul(s2[:], ones_t[:], sq[:, 0, :], start=True, stop=False)
                nc.tensor.matmul(s2[:], ones_t[:], sq[:, 1, :], start=False, stop=True)
                mu2 = sb2.tile([128, TN], BF, tag="mu2", name="mu2" + nm)
                nc.scalar.activation(mu2[:], s1[:], AF.Square, scale=1.0 / C)
                mu_b = sb2.tile([128, TN], BF, tag="mu_b", name="mu_b" + nm)
                nc.scalar.activation(mu_b[:], s1[:], AF.Identity, scale=1.0 / C)
                var = sb2.tile([128, TN], F32, tag="var", name="var" + nm)
                nc.vector.scalar_tensor_tensor(var[:], s2[:], 1.0 / C, mu2[:],
                                               op0=AL.mult, op1=AL.subtract)
                nc.scalar.activation(var[:], var[:], AF.Ln, bias=epsb[:])
                rl = sb2.tile([128, TN], BF, tag="rl", name="rl" + nm)
                nc.scalar.activation(rl[:], var[:], AF.Exp, scale=-0.5)
                m2 = sb2.tile([128, TN], BF, tag="m2", name="m2" + nm)
                nc.vector.tensor_tensor(m2[:], mu_b[:], rl[:], op=AL.mult)
                return rl, m2

            def bq_dma(t):
                sl = slice(t * TN, (t + 1) * TN)
                lo_t = lop.tile([128, 2, TN], BF, tag="lo", name="lo_t")
                nc.sync.dma_start(lo_t[:], lo16[:, :, sl])
                lo_a[t] = lo_t

            def bq_sq(t):
                lo_t = lo_a[t]
                sq = sb2.tile([128, 2, TN], BF, tag="sq", name="sqq", bufs=3)
                nc.gpsimd.tensor_tensor(sq[:, 0, :], lo_t[:, 0, :], lo_t[:, 0, :], op=AL.mult)
                nc.gpsimd.tensor_tensor(sq[:, 1, :], lo_t[:, 1, :], lo_t[:, 1, :], op=AL.mult)
                sq_a[t] = sq

            def build_query(t):
                lo_t = lo_a[t]
                rl, m2 = ln_tile(lo_t[:, 0, :], lo_t[:, 1, :], "q", sq=sq_a.pop(t))
                qt = qp.tile([128, 2, 2 * QH + TN], BF, tag="qt")
                qtiles[t] = qt
                for ct in range(2):
                    nc.gpsimd.tensor_tensor(qt[:, ct, QH:QH + TN], lo_t[:, ct, :], rl[:],
                                            op=AL.mult)
                for ct in range(2):
                    nc.vector.tensor_tensor(qt[:, ct, QH:QH + TN], qt[:, ct, QH:QH + TN],
                                            m2[:], op=AL.subtract)
                if t == 0:
                    nc.vector.memset(qt[:, :, 0:QH], 0.0)
                else:
                    for ct in range(2):
                        nc.vector.tensor_copy(qt[:, ct, 0:QH],
                                              qtiles[t - 1][:, ct, TN:TN + QH])
                        nc.vector.tensor_copy(qtiles[t - 1][:, ct, QH + TN:],
                                              qt[:, ct, QH:2 * QH])
                if t == S2_T - 1:
                    nc.vector.memset(qt[:, :, QH + TN:], 0.0)

            def dw9(psum, wtile, src):
                for ti, (dr, dc) in enumerate(TAP_ORDER):
                    tap = (dr + 1) * 3 + (dc + 1)
                    off0 = QH + dr * 128
                    lhs = wtile[:, tap, :]
                    if dc == 0:
                        nc.tensor.matmul(psum[:], lhs, src[:, off0:off0 + TN],
                                         start=(ti == 0), stop=(ti == 8))
                    else:
                        rhs3 = src[:, off0:off0 + TN].rearrange("p (r w) -> p r w", w=128)
                        out3 = psum[:].rearrange("p (r w) -> p r w", w=128)
                        if dc == -1:
                            nc.tensor.matmul(out3[:, :, 1:128], lhs, rhs3[:, :, 0:127],
                                             start=False, stop=(ti == 8))
                        else:
                            nc.tensor.matmul(out3[:, :, 0:127], lhs, rhs3[:, :, 1:128],
                                             start=False, stop=(ti == 8))

            def attn_qdw(s, ct):
                qt = qtiles[s]
                if ct == 0:
                    qd = sb2.tile([128, 2, TN], BF, tag="qd")
                    qd_a[s] = qd
                qd = qd_a[s]
                qdp = ps.tile([128, TN], F32, tag="mm", name="qdp", bufs=3)
                dw9(qdp, wqdw_t[:, ct], qt[:, ct, :])
                nc.vector.tensor_copy(qd[:, ct, :], qdp[:])

            def attn_qk2(s, hf):
                qd = qd_a[s]
                if hf == 0:
                    e_a[s] = []
                    rz_a[s] = sb2.tile([4, 2, TN], F32, tag="rz", bufs=2, name="rz")
                lp = ps.tile([128, TN], F32, tag="at", name="lp", bufs=1)
                for kt in range(2):
                    nc.tensor.matmul(lp[0:76, :], kbd_t[:, kt, hf * 76:hf * 76 + 76],
                                     qd[:, kt, :], start=(kt == 0), stop=(kt == 1))
                e_h = sb2.tile([76, TN], BF, tag="eh%d" % hf)
                nc.scalar.activation(e_h[:], lp[0:76, :], AF.Exp, scale=-SCALE,
                                     bias=bexp_t[0:76, hf:hf + 1])
                e_a[s].append(e_h)
                zp_ = ps.tile([4, TN], F32, tag="at", name="zp_", bufs=1)
                nc.tensor.matmul(zp_[:], obd_t[0:76, :], e_h[:], start=True, stop=True)
                nc.vector.reciprocal_approx_fast(rz_a[s][:, hf, :], zp_[:])
                if hf == 1:
                    del qd_a[s]

            def attn_en(s, hf):
                e_ab = e_a[s]
                rz = rz_a[s]
                rzx = ps.tile([128, TN], F32, tag="at", name="rzx", bufs=1)
                nc.tensor.matmul(rzx[0:76, :], expd_t[:], rz[:, hf, :],
                                 start=True, stop=True)
                en = sb2.tile([76, TN], BF, tag="en%d" % hf, name="en")
                nc.vector.tensor_tensor(en[:], e_ab[hf][:], rzx[0:76, :], op=AL.mult)
                en_a.setdefault(s, []).append(en)
                if hf == 1:
                    del e_a[s]
                    del rz_a[s]

            def attn_proj(s, mt):
                en = en_a[s]
                if mt == 0:
                    out_t = outp.tile([128, 2, TN], BF, tag="out")
                    out_a[s] = out_t
                out_t = out_a[s]
                op_ = ps.tile([128, TN], F32, tag="mm", name="op_", bufs=3)
                for hf in range(2):
                    nc.tensor.matmul(op_[:], pvbd_t[0:76, hf, mt * 128:(mt + 1) * 128],
                                     en[hf][:], start=(hf == 0), stop=(hf == 1))
                nc.vector.scalar_tensor_tensor(out_t[:, mt, :], op_[:],
                                               bprj_t[:, mt:mt + 1],
                                               lo_a[s][:, mt, :], op0=AL.add, op1=AL.add)
                if mt == 1:
                    del en_a[s]
                    del lo_a[s]

            def attn_ln(s):
                out_t = out_a[s]
                ro, m2o = ln_tile(out_t[:, 0, :], out_t[:, 1, :], "o")
                yl_t = ylp.tile([128, 2, TN], BF, tag="yl")
                yl_a[s] = yl_t
                for ct in range(2):
                    nc.gpsimd.tensor_tensor(yl_t[:, ct, :], out_t[:, ct, :], ro[:],
                                            op=AL.mult)
                for ct in range(2):
                    nc.vector.tensor_tensor(yl_t[:, ct, :], yl_t[:, ct, :], m2o[:],
                                            op=AL.subtract)

            def build_z_pre(t):
                zt = zp.tile([128, 8, 2 * QH + TN], BF, tag="zt")
                ztiles[t] = zt

            def build_z_g(t, g):
                yl_t = yl_a[t]
                zt = ztiles[t]
                m1p = ps.tile([128, TN], F32, tag="mm", name="m1p", bufs=3)
                for kt in range(2):
                    nc.tensor.matmul(m1p[:], wmlp1_t[:, kt, g * 128:(g + 1) * 128],
                                     yl_t[:, kt, :], start=(kt == 0), stop=(kt == 1))
                if g % 2 == 0:
                    nc.scalar.activation(zt[:, g, QH:QH + TN], m1p[:], AF.Identity,
                                         bias=b1_t[:, g:g + 1])
                else:
                    nc.vector.tensor_scalar(zt[:, g, QH:QH + TN], m1p[:],
                                            b1_t[:, g:g + 1], None, op0=AL.add)
                if t == 0:
                    nc.vector.tensor_tensor(zt[:, g, QH:QH + TN], zt[:, g, QH:QH + TN],
                                            zm_t[:, 0, :], op=AL.mult)
                elif t == S2_T - 1:
                    nc.vector.tensor_tensor(zt[:, g, QH:QH + TN], zt[:, g, QH:QH + TN],
                                            zm_t[:, 1, :], op=AL.mult)

            def build_z_post(t):
                zt = ztiles[t]
                del yl_a[t]
                if t == 0:
                    nc.vector.memset(zt[:, :, 0:QH], 0.0)
                else:
                    for g in range(8):
                        nc.vector.tensor_copy(zt[:, g, 0:QH],
                                              ztiles[t - 1][:, g, TN:TN + QH])
                        nc.vector.tensor_copy(ztiles[t - 1][:, g, QH + TN:],
                                              zt[:, g, QH:2 * QH])
                if t == S2_T - 1:
                    nc.vector.memset(zt[:, :, QH + TN:], 0.0)

            def ffn_g(s, g):
                zt = ztiles[s]
                if g == 0:
                    f0 = ps.tile([128, TN], F32, tag="f01", name="f0")
                    f1 = ps.tile([128, TN], F32, tag="f01", name="f1")
                    f01_a[s] = (f0, f1)
                f0, f1 = f01_a[s]
                dwp = ps.tile([128, TN], F32, tag="mm", name="dwp", bufs=3)
                dw9(dwp, wdwm_t[:, g], zt[:, g, :])
                gel = sb2.tile([128, TN], BF, tag="gel")
                nc.scalar.activation(gel[:], dwp[:], AF.Gelu, bias=bdw_t[:, g:g + 1])
                nc.tensor.matmul(f0[:], wmlp2_t[:, g, 0:128], gel[:],
                                 start=(g == 0), stop=(g == 7))
                nc.tensor.matmul(f1[:], wmlp2_t[:, g, 128:256], gel[:],
                                 start=(g == 0), stop=(g == 7))

            def ffn_fin(s):
                f0, f1 = f01_a.pop(s)
                if s == 0:
                    px0, px1, o0 = 256, TN, 0
                elif s == S2_T - 1:
                    px0, px1, o0 = 0, 256, (S2_T - 1) * TN - 256
                else:
                    px0, px1, o0 = 0, TN, s * TN - 256
                n = px1 - px0
                for ct, fps in enumerate((f0, f1)):
                    fin = sb2.tile([128, TN], F32, tag="fin", name="fin")
                    nc.vector.scalar_tensor_tensor(fin[:, 0:n], fps[:, px0:px1],
                                                   b2_t[:, ct:ct + 1],
                                                   out_a[s][:, ct, px0:px1],
                                                   op0=AL.add, op1=AL.add)
                    nc.sync.dma_start(OUT[:, ct, o0:o0 + n], fin[:, 0:n])
                del out_a[s]

            # software pipeline with fine-grained PE-stream interleave:
            # z(t-3) mlp1 groups zip with the query-conv of attn(t-2); the
            # ffn(t-4) dw groups zip with the attention softmax/AV/proj chain
            # so the in-order PE queue never waits on an ACT/DVE consumer.
            for t in range(S2_T + 4):
                bz = 3 <= t < S2_T + 3      # build_z(t-3)
                qk = 2 <= t < S2_T + 2      # attn(t-2)
                fn = 4 <= t < S2_T + 4      # ffn(t-4)
                if bz:
                    build_z_pre(t - 3)
                    build_z_g(t - 3, 0)
                    build_z_g(t - 3, 1)
                if qk:
                    attn_qdw(t - 2, 0)
                if bz:
                    build_z_g(t - 3, 2)
                    build_z_g(t - 3, 3)
                if qk:
                    attn_qdw(t - 2, 1)
                if bz:
                    build_z_g(t - 3, 4)
                    build_z_g(t - 3, 5)
                if qk:
                    attn_qpw(t - 2)
                if bz:
                    build_z_g(t - 3, 6)
                    build_z_g(t - 3, 7)
                    build_z_post(t - 3)
                if qk:
                    attn_qk2(t - 2, 0)
                if fn:
                    ffn_g(t - 4, 0)
                if qk:
                    attn_qk2(t - 2, 1)
                if fn:
                    ffn_g(t - 4, 1)
                if qk:
                    attn_av1(t - 2, 0)
                if fn:
                    ffn_g(t - 4, 2)
                if qk:
                    attn_av1(t - 2, 1)
                if fn:
                    ffn_g(t - 4, 3)
                if qk:
                    attn_proj(t - 2, 0)
                if fn:
                    ffn_g(t - 4, 4)
                if qk:
                    attn_proj(t - 2, 1)
                if fn:
                    ffn_g(t - 4, 5)
                if qk:
                    attn_ln(t - 2)
                if fn:
                    ffn_g(t - 4, 6)
                    ffn_g(t - 4, 7)
                    ffn_fin(t - 4)
                    del ztiles[t - 4]
                if t < S2_T:
                    build_query(t)
                if qk:
                    del qtiles[t - 2]

    nc.finalize()
    return nc


# ----------------------------------------------------------------------------
# stage 2 builder
# ----------------------------------------------------------------------------

def build_stage2():
    """Fully fused stage 2: one software-pipelined loop per tile.

    iter t: build_z(t-3) | attn(t-2) | ffn(t-4) | build_query(t), with the
    ffn dw-conv groups interleaved into the attention chain so the in-order
    PE queue always has independent matmul work. Per-tile LN stats via
    ones-matmul broadcast + Ln/Exp (combined act table). The query 1x1 conv
    is folded into K (kbd = kq) and the output projection into V (pvbd), so
    q/av intermediates never materialize.
    """
    nc = bacc.Bacc()
    NPX = S2_T * TN
    lo16 = nc.dram_tensor("lo16", [128, 2, NPX], BF, kind="ExternalInput")
    ones = nc.dram_tensor("ones", [128, 128], BF, kind="ExternalInput")
    kbd = nc.dram_tensor("kbd", [128, 2, 152], BF, kind="ExternalInput")
    pvbd = nc.dram_tensor("pvbd", [128, 2, 256], BF, kind="ExternalInput")
    onesbd = nc.dram_tensor("onesbd", [128, 4], BF, kind="ExternalInput")
    expd = nc.dram_tensor("expd", [4, 76], F32, kind="ExternalInput")
    bexp = nc.dram_tensor("bexp", [128, 2], F32, kind="ExternalInput")
    wqdw = nc.dram_tensor("wqdw", [128, 2, 9, 128], BF, kind="ExternalInput")
    wmlp1 = nc.dram_tensor("wmlp1", [128, 2, 1024], BF, kind="ExternalInput")
    wdwm = nc.dram_tensor("wdwm", [128, 8, 9, 128], BF, kind="ExternalInput")
    wmlp2 = nc.dram_tensor("wmlp2", [128, 8, 256], BF, kind="ExternalInput")
    bprj = nc.dram_tensor("bprj", [128, 2], F32, kind="ExternalInput")
    b1 = nc.dram_tensor("b1", [128, 8], F32, kind="ExternalInput")
    bdw = nc.dram_tensor("bdw", [128, 8], F32, kind="ExternalInput")
    b2 = nc.dram_tensor("b2", [128, 2], F32, kind="ExternalInput")
    zmask = nc.dram_tensor("zmask", [128, 2, TN], BF, kind="ExternalInput")
    OUT = nc.dram_tensor("OUT", [128, 2, S1_T * TN], F32, kind="ExternalOutput")

    with TileContext(nc) as tc:
        with (
            tc.tile_pool(name="cst", bufs=1) as cst,
            tc.tile_pool(name="lop", bufs=5) as lop,
            tc.tile_pool(name="sb2", bufs=3) as sb2,
            tc.tile_pool(name="qp", bufs=4) as qp,
            tc.tile_pool(name="zp", bufs=4) as zp,
            tc.tile_pool(name="outp", bufs=5) as outp,
            tc.tile_pool(name="ylp", bufs=3) as ylp,
            tc.tile_pool(name="ps", bufs=2, space="PSUM") as ps,
        ):
            ones_t = cst.tile([128, 128], BF, tag="ones"); nc.sync.dma_start(ones_t[:], ones[:])
            kbd_t = cst.tile([128, 2, 152], BF, tag="kbd"); nc.sync.dma_start(kbd_t[:], kbd[:])
            pvbd_t = cst.tile([128, 2, 256], BF, tag="pvbd"); nc.sync.dma_start(pvbd_t[:], pvbd[:])
            obd_t = cst.tile([128, 4], BF, tag="obd"); nc.sync.dma_start(obd_t[:], onesbd[:])
            expd_t = cst.tile([4, 76], F32, tag="expd"); nc.sync.dma_start(expd_t[:], expd[:])
            bexp_t = cst.tile([128, 2], F32, tag="bexp"); nc.sync.dma_start(bexp_t[:], bexp[:])
            wqdw_t = cst.tile([128, 2, 9, 128], BF, tag="wqdw"); nc.sync.dma_start(wqdw_t[:], wqdw[:])
            wmlp1_t = cst.tile([128, 2, 1024], BF, tag="wmlp1"); nc.sync.dma_start(wmlp1_t[:], wmlp1[:])
            wdwm_t = cst.tile([128, 8, 9, 128], BF, tag="wdwm"); nc.sync.dma_start(wdwm_t[:], wdwm[:])
            wmlp2_t = cst.tile([128, 8, 256], BF, tag="wmlp2"); nc.sync.dma_start(wmlp2_t[:], wmlp2[:])
            bprj_t = cst.tile([128, 2], F32, tag="bprj"); nc.sync.dma_start(bprj_t[:], bprj[:])
            b1_t = cst.tile([128, 8], F32, tag="b1"); nc.sync.dma_start(b1_t[:], b1[:])
            bdw_t = cst.tile([128, 8], F32, tag="bdw"); nc.sync.dma_start(bdw_t[:], bdw[:])
            b2_t = cst.tile([128, 2], F32, tag="b2"); nc.sync.dma_start(b2_t[:], b2[:])
            zm_t = cst.tile([128, 2, TN], BF, tag="zm"); nc.sync.dma_start(zm_t[:], zmask[:])
            epsb = cst.tile([128, 1], F32, tag="epsb")
            nc.vector.memset(epsb[:], EPS)

            qtiles = {}
            ztiles = {}
            lo_a = {}
            out_a = {}
            yl_a = {}
            e_a = {}
            rz_a = {}
            f01_a = {}
            qd_a = {}
            en_a = {}
            sq_a = {}

            # per-tile LN stats -> (rstd, mu*rstd) bf16 full-width tiles
            def ln_tile(x0, x1, nm, sq=None):
                if sq is None:
                    sq = sb2.tile([128, 2, TN], BF, tag="sq", name="sq" + nm, bufs=3)
                    nc.gpsimd.tensor_tensor(sq[:, 0, :], x0, x0, op=AL.mult)
                    nc.gpsimd.tensor_tensor(sq[:, 1, :], x1, x1, op=AL.mult)
                s1 = ps.tile([128, TN], F32, tag="st", name="s1" + nm)
                nc.tensor.matmul(s1[:], ones_t[:], x0, start=True, stop=False)
                nc.tensor.matmul(s1[:], ones_t[:], x1, start=False, stop=True)
                s2 = ps.tile([128, TN], F32, tag="st", name="s2" + nm)
                nc.tensor.matmul(s2[:], ones_t[:], sq[:, 0, :], start=True, stop=False)
                nc.tensor.matmul(s2[:], ones_t[:], sq[:, 1, :], start=False, stop=True)
                mu2 = sb2.tile([128, TN], BF, tag="mu2", name="mu2" + nm)
                nc.scalar.activation(mu2[:], s1[:], AF.Square, scale=1.0 / C)
                mu_b = sb2.tile([128, TN], BF, tag="mu_b", name="mu_b" + nm)
                nc.scalar.activation(mu_b[:], s1[:], AF.Identity, scale=1.0 / C)
                var = sb2.tile([128, TN], F32, tag="var", name="var" + nm)
                nc.vector.scalar_tensor_tensor(var[:], s2[:], 1.0 / C, mu2[:],
                                               op0=AL.mult, op1=AL.subtract)
                nc.scalar.activation(var[:], var[:], AF.Ln, bias=epsb[:])
                rl = sb2.tile([128, TN], BF, tag="rl", name="rl" + nm)
                nc.scalar.activation(rl[:], var[:], AF.Exp, scale=-0.5)
                m2 = sb2.tile([128, TN], BF, tag="m2", name="m2" + nm)
                nc.vector.tensor_tensor(m2[:], mu_b[:], rl[:], op=AL.mult)
                return rl, m2

            def bq_dma(t):
                sl = slice(t * TN, (t + 1) * TN)
                lo_t = lop.tile([128, 2, TN], BF, tag="lo", name="lo_t")
                nc.sync.dma_start(lo_t[:], lo16[:, :, sl])
                lo_a[t] = lo_t

            def bq_sq(t):
                lo_t = lo_a[t]
                sq = sb2.tile([128, 2, TN], BF, tag="sq", name="sqq", bufs=3)
                nc.gpsimd.tensor_tensor(sq[:, 0, :], lo_t[:, 0, :], lo_t[:, 0, :], op=AL.mult)
                nc.gpsimd.tensor_tensor(sq[:, 1, :], lo_t[:, 1, :], lo_t[:, 1, :], op=AL.mult)
                sq_a[t] = sq

            def build_query(t):
                lo_t = lo_a[t]
                rl, m2 = ln_tile(lo_t[:, 0, :], lo_t[:, 1, :], "q", sq=sq_a.pop(t))
                qt = qp.tile([128, 2, 2 * QH + TN], BF, tag="qt")
                qtiles[t] = qt
                for ct in range(2):
                    nc.gpsimd.tensor_tensor(qt[:, ct, QH:QH + TN], lo_t[:, ct, :], rl[:],
                                            op=AL.mult)
                for ct in range(2):
                    nc.vector.tensor_tensor(qt[:, ct, QH:QH + TN], qt[:, ct, QH:QH + TN],
                                            m2[:], op=AL.subtract)
                if t == 0:
                    nc.vector.memset(qt[:, :, 0:QH], 0.0)
                else:
                    for ct in range(2):
                        nc.vector.tensor_copy(qt[:, ct, 0:QH],
                                              qtiles[t - 1][:, ct, TN:TN + QH])
                        nc.vector.tensor_copy(qtiles[t - 1][:, ct, QH + TN:],
                                              qt[:, ct, QH:2 * QH])
                if t == S2_T - 1:
                    nc.vector.memset(qt[:, :, QH + TN:], 0.0)

            def dw9(psum, wtile, src):
                for ti, (dr, dc) in enumerate(TAP_ORDER):
                    tap = (dr + 1) * 3 + (dc + 1)
                    off0 = QH + dr * 128
                    lhs = wtile[:, tap, :]
                    if dc == 0:
                        nc.tensor.matmul(psum[:], lhs, src[:, off0:off0 + TN],
                                         start=(ti == 0), stop=(ti == 8))
                    else:
                        rhs3 = src[:, off0:off0 + TN].rearrange("p (r w) -> p r w", w=128)
                        out3 = psum[:].rearrange("p (r w) -> p r w", w=128)
                        if dc == -1:
                            nc.tensor.matmul(out3[:, :, 1:128], lhs, rhs3[:, :, 0:127],
                                             start=False, stop=(ti == 8))
                        else:
                            nc.tensor.matmul(out3[:, :, 0:127], lhs, rhs3[:, :, 1:128],
                                             start=False, stop=(ti == 8))

            def attn_qdw(s, ct):
                qt = qtiles[s]
                if ct == 0:
                    qd = sb2.tile([128, 2, TN], BF, tag="qd")
                    qd_a[s] = qd
                qd = qd_a[s]
                qdp = ps.tile([128, TN], F32, tag="mm", name="qdp", bufs=3)
                dw9(qdp, wqdw_t[:, ct], qt[:, ct, :])
                nc.vector.tensor_copy(qd[:, ct, :], qdp[:])

            def attn_qk2(s, hf):
                qd = qd_a[s]
                if hf == 0:
                    e_a[s] = []
                    rz_a[s] = sb2.tile([4, 2, TN], F32, tag="rz", bufs=2, name="rz")
                lp = ps.tile([128, TN], F32, tag="at", name="lp", bufs=1)
                for kt in range(2):
                    nc.tensor.matmul(lp[0:76, :], kbd_t[:, kt, hf * 76:hf * 76 + 76],
                                     qd[:, kt, :], start=(kt == 0), stop=(kt == 1))
                e_h = sb2.tile([76, TN], BF, tag="eh%d" % hf)
                nc.scalar.activation(e_h[:], lp[0:76, :], AF.Exp, scale=-SCALE,
                                     bias=bexp_t[0:76, hf:hf + 1])
                e_a[s].append(e_h)
                zp_ = ps.tile([4, TN], F32, tag="at", name="zp_", bufs=1)
                nc.tensor.matmul(zp_[:], obd_t[0:76, :], e_h[:], start=True, stop=True)
                nc.vector.reciprocal_approx_fast(rz_a[s][:, hf, :], zp_[:])
                if hf == 1:
                    del qd_a[s]

            def attn_en(s, hf):
                e_ab = e_a[s]
                rz = rz_a[s]
                rzx = ps.tile([128, TN], F32, tag="at", name="rzx", bufs=1)
                nc.tensor.matmul(rzx[0:76, :], expd_t[:], rz[:, hf, :],
                                 start=True, stop=True)
                en = sb2.tile([76, TN], BF, tag="en%d" % hf, name="en")
                nc.vector.tensor_tensor(en[:], e_ab[hf][:], rzx[0:76, :], op=AL.mult)
                en_a.setdefault(s, []).append(en)
                if hf == 1:
                    del e_a[s]
                    del rz_a[s]

            def attn_proj(s, mt):
                en = en_a[s]
                if mt == 0:
                    out_t = outp.tile([128, 2, TN], BF, tag="out")
                    out_a[s] = out_t
                out_t = out_a[s]
                op_ = ps.tile([128, TN], F32, tag="mm", name="op_", bufs=3)
                for hf in range(2):
                    nc.tensor.matmul(op_[:], pvbd_t[0:76, hf, mt * 128:(mt + 1) * 128],
                                     en[hf][:], start=(hf == 0), stop=(hf == 1))
                nc.vector.scalar_tensor_tensor(out_t[:, mt, :], op_[:],
                                               bprj_t[:, mt:mt + 1],
                                               lo_a[s][:, mt, :], op0=AL.add, op1=AL.add)
                if mt == 1:
                    del en_a[s]
                    del lo_a[s]

            def attn_ln(s):
                out_t = out_a[s]
                ro, m2o = ln_tile(out_t[:, 0, :], out_t[:, 1, :], "o")
                yl_t = ylp.tile([128, 2, TN], BF, tag="yl")
                yl_a[s] = yl_t
                for ct in range(2):
                    nc.gpsimd.tensor_tensor(yl_t[:, ct, :], out_t[:, ct, :], ro[:],
                                            op=AL.mult)
                for ct in range(2):
                    nc.vector.tensor_tensor(yl_t[:, ct, :], yl_t[:, ct, :], m2o[:],
                                            op=AL.subtract)

            def build_z_pre(t):
                zt = zp.tile([128, 8, 2 * QH + TN], BF, tag="zt")
                ztiles[t] = zt

            def build_z_g(t, g):
                yl_t = yl_a[t]
                zt = ztiles[t]
                m1p = ps.tile([128, TN], F32, tag="mm", name="m1p", bufs=3)
                for kt in range(2):
                    nc.tensor.matmul(m1p[:], wmlp1_t[:, kt, g * 128:(g + 1) * 128],
                                     yl_t[:, kt, :], start=(kt == 0), stop=(kt == 1))
                if g % 2 == 0:
                    nc.scalar.activation(zt[:, g, QH:QH + TN], m1p[:], AF.Identity,
                                         bias=b1_t[:, g:g + 1])
                else:
                    nc.vector.tensor_scalar(zt[:, g, QH:QH + TN], m1p[:],
                                            b1_t[:, g:g + 1], None, op0=AL.add)
                if t == 0:
                    nc.vector.tensor_tensor(zt[:, g, QH:QH + TN], zt[:, g, QH:QH + TN],
                                            zm_t[:, 0, :], op=AL.mult)
                elif t == S2_T - 1:
                    nc.vector.tensor_tensor(zt[:, g, QH:QH + TN], zt[:, g, QH:QH + TN],
                                            zm_t[:, 1, :], op=AL.mult)

            def build_z_post(t):
                zt = ztiles[t]
                del yl_a[t]
                if t == 0:
                    nc.vector.memset(zt[:, :, 0:QH], 0.0)
                else:
                    for g in range(8):
                        nc.vector.tensor_copy(zt[:, g, 0:QH],
                                              ztiles[t - 1][:, g, TN:TN + QH])
                        nc.vector.tensor_copy(ztiles[t - 1][:, g, QH + TN:],
                                              zt[:, g, QH:2 * QH])
                if t == S2_T - 1:
                    nc.vector.memset(zt[:, :, QH + TN:], 0.0)

            def ffn_g(s, g):
                zt = ztiles[s]
                if g == 0:
                    f0 = ps.tile([128, TN], F32, tag="f01", name="f0")
                    f1 = ps.tile([128, TN], F32, tag="f01", name="f1")
                    f01_a[s] = (f0, f1)
                f0, f1 = f01_a[s]
                dwp = ps.tile([128, TN], F32, tag="mm", name="dwp", bufs=3)
                dw9(dwp, wdwm_t[:, g], zt[:, g, :])
                gel = sb2.tile([128, TN], BF, tag="gel")
                nc.scalar.activation(gel[:], dwp[:], AF.Gelu, bias=bdw_t[:, g:g + 1])
                nc.tensor.matmul(f0[:], wmlp2_t[:, g, 0:128], gel[:],
                                 start=(g == 0), stop=(g == 7))
                nc.tensor.matmul(f1[:], wmlp2_t[:, g, 128:256], gel[:],
                                 start=(g == 0), stop=(g == 7))

            def ffn_fin(s):
                f0, f1 = f01_a.pop(s)
                if s == 0:
                    px0, px1, o0 = 256, TN, 0
                elif s == S2_T - 1:
                    px0, px1, o0 = 0, 256, (S2_T - 1) * TN - 256
                else:
                    px0, px1, o0 = 0, TN, s * TN - 256
                n = px1 - px0
                for ct, fps in enumerate((f0, f1)):
                    fin = sb2.tile([128, TN], F32, tag="fin", name="fin")
                    nc.vector.scalar_tensor_tensor(fin[:, 0:n], fps[:, px0:px1],
                                                   b2_t[:, ct:ct + 1],
                                                   out_a[s][:, ct, px0:px1],
                                                   op0=AL.add, op1=AL.add)
                    nc.sync.dma_start(OUT[:, ct, o0:o0 + n], fin[:, 0:n])
                del out_a[s]

            # software pipeline: z(t-3) | attn(t-2) | ffn(t-4) | query(t);
            # ffn dw groups interleave into the attention chain so the
            # in-order PE queue always has independent matmul work.
            for t in range(S2_T + 4):
                bz = 3 <= t < S2_T + 3      # build_z(t-3)
                qk = 2 <= t < S2_T + 2      # attn(t-2)
                fn = 4 <= t < S2_T + 4      # ffn(t-4)
                if bz:
                    build_z_pre(t - 3)
                    for g in range(8):
                        build_z_g(t - 3, g)
                    build_z_post(t - 3)
                if qk:
                    attn_qdw(t - 2, 0)
                    attn_qdw(t - 2, 1)
                    # both attention Exps back-to-back: exactly one act-table
                    # switch into Gelu below and one back per iteration
                    attn_qk2(t - 2, 0)
                    attn_qk2(t - 2, 1)
                if fn:
                    for g in range(8):
                        ffn_g(t - 4, g)
                if qk:
                    attn_en(t - 2, 0)
                    attn_en(t - 2, 1)
                    attn_proj(t - 2, 0)
                    attn_proj(t - 2, 1)
                if fn:
                    ffn_fin(t - 4)
                    del ztiles[t - 4]
                if qk:
                    attn_ln(t - 2)
                if t < S2_T:
                    build_query(t)
                if qk:
                    del qtiles[t - 2]

    nc.finalize()
    return nc


# ----------------------------------------------------------------------------
# host packing
# ----------------------------------------------------------------------------

def _chunk(x, b, r0, r1, pad_lo, pad_hi):
    """x[b] rows [r0-pad_lo, r1+pad_hi) zero-clamped -> [128, 2, n*128]."""
    lo_pad = np.zeros((C, pad_lo, W), np.float32)
    hi_pad = np.zeros((C, pad_hi, W), np.float32)
    lo_src = x[b, :, max(r0 - pad_lo, 0):r0, :]
    if lo_src.shape[1] > 0:
        lo_pad[:, pad_lo - lo_src.shape[1]:, :] = lo_src
    hi_src = x[b, :, r1:min(r1 + pad_hi, H), :]
    if hi_src.shape[1] > 0:
        hi_pad[:, :hi_src.shape[1], :] = hi_src
    full = np.concatenate([lo_pad, np.asarray(x[b, :, r0:r1, :], np.float32), hi_pad], axis=1)
    n = full.shape[1]
    return np.ascontiguousarray(full.reshape(2, 128, n * W).transpose(1, 0, 2))


def _bcast_rowsel():
    m = np.zeros((128, 4 * 128), np.float32)
    for i, r in enumerate((0, 32, 64, 96)):
        m[r, i * 128:(i + 1) * 128] = 1.0
    return m.astype(bf16)


_S1 = None
_S2 = None
_last_s1_inputs = None
_last_s2_inputs = None


def kernel(**inp):
    global _S1, _S2
    f32 = np.float32
    low = np.asarray(inp["low"], f32)
    high = np.asarray(inp["high"], f32)
    g_low = np.asarray(inp["g_low"], f32); b_low = np.asarray(inp["b_low"], f32)
    g_high = np.asarray(inp["g_high"], f32); b_high = np.asarray(inp["b_high"], f32)
    g_mlp = np.asarray(inp["g_mlp"], f32); b_mlp = np.asarray(inp["b_mlp"], f32)
    w_q_dw = np.asarray(inp["w_q_dw"], f32); b_q_dw = np.asarray(inp["b_q_dw"], f32)
    w_q_pw = np.asarray(inp["w_q_pw"], f32)[:, :, 0, 0]; b_q_pw = np.asarray(inp["b_q_pw"], f32)
    w_ml1 = np.asarray(inp["w_ml1"], f32)[:, :, 0, 0]
    w_ml2 = np.asarray(inp["w_ml2"], f32)[:, :, 0, 0]
    w_align = np.asarray(inp["w_align"], f32)[:, :, 0, 0]
    w_kv = np.asarray(inp["w_kv"], f32); b_kv = np.asarray(inp["b_kv"], f32)
    memory = np.asarray(inp["memory"], f32)
    w_proj = np.asarray(inp["w_proj"], f32)[:, :, 0, 0]; b_proj = np.asarray(inp["b_proj"], f32)
    w_mlp1 = np.asarray(inp["w_mlp1"], f32)[:, :, 0, 0]; b_mlp1 = np.asarray(inp["b_mlp1"], f32)
    w_mlp_dw = np.asarray(inp["w_mlp_dw"], f32); b_mlp_dw = np.asarray(inp["b_mlp_dw"], f32)
    w_mlp2 = np.asarray(inp["w_mlp2"], f32)[:, :, 0, 0]; b_mlp2 = np.asarray(inp["b_mlp2"], f32)

    assert np.allclose(g_low, g_high), "kernel requires g_low == g_high"

    def dense_grouped(wg, groups):
        o, ipg = wg.shape
        d = np.zeros((o, ipg * groups), f32)
        opg = o // groups
        for g in range(groups):
            d[g * opg:(g + 1) * opg, g * ipg:(g + 1) * ipg] = wg[g * opg:(g + 1) * opg]
        return d

    Wm1 = dense_grouped(w_ml1, 4)
    Wal = dense_grouped(w_align, 4)
    Wm1g = Wm1 * g_low[None, :]
    Walg = Wal * g_low[None, :]
    bb = (b_low + b_high) * 0.5
    xa_bias = Wal @ bb
    ones128 = np.ones((128, 128), f32)
    ident = np.eye(128, dtype=f32)

    def pf(x):  # [k, ...] stacked lhsT tiles -> partition-first
        return np.ascontiguousarray(np.moveaxis(x, 1, 0)) if False else x

    wm1_h = np.ascontiguousarray(
        np.stack([Wm1g.T[0:128, 0:128], Wm1g.T[128:256, 128:256]]).transpose(1, 0, 2)).astype(bf16)
    wm2_h = np.ascontiguousarray(
        np.stack([w_ml2.T[0:128], w_ml2.T[128:256]]).transpose(1, 0, 2)).astype(bf16)
    # walT is the rhs layout for the px-partition xa^T matmul:
    # walT[p, ct, c'] = Walg[c', ct*128 + p]
    walT_h = np.ascontiguousarray(
        Walg.T.reshape(2, 128, 256).transpose(1, 0, 2)).astype(bf16)

    s1_core = []
    for core in range(NCORES):
        b, hf = core // 2, core % 2
        r0 = hf * R
        s1_core.append(dict(
            lo=_chunk(low, b, r0, r0 + R, 0, 0).astype(bf16),
            hi=_chunk(high, b, r0, r0 + R, 0, 0).astype(bf16),
            ones=ones128.astype(bf16),
            wm1=wm1_h, wm2=wm2_h, walT=walT_h,
        ))

    global _last_s1_inputs
    _last_s1_inputs = s1_core
    if _S1 is None:
        _S1 = build_stage1()
    res1 = run_bass_kernel_spmd(_S1, s1_core, core_ids=list(range(NCORES)))

    S = np.zeros((B, NCL, 256), f32)
    Z = np.zeros((B, NCL), f32)
    for core in range(NCORES):
        b = core // 2
        S[b] += res1.results[core]["S_out"]
        Z[b] += res1.results[core]["Z_out"][:, 0]
    cf = S / Z[:, :, None] + xa_bias[None, None, :]
    cf = (1.0 - MOM) * cf + MOM * memory
    kv = cf @ w_kv.T + b_kv
    k, v = kv[:, :, :256], kv[:, :, 256:]

    # folded q-path biases -> per (b, head, class) logit bias
    cb1 = b_low * w_q_dw[:, 0].sum(axis=(1, 2)) + b_q_dw
    cb2 = w_q_pw @ cb1 + b_q_pw
    lbh = np.zeros((B, NH, NCL), f32)
    for h in range(NH):
        lbh[:, h, :] = np.einsum("bnd,d->bn", k[:, :, 32 * h:32 * h + 32],
                                 cb2[32 * h:32 * h + 32])

    wqdw_diag = np.zeros((2, 9, 128, 128), f32)
    wdw_g = w_q_dw[:, 0] * g_low[:, None, None]
    for ct in range(2):
        for tap in range(9):
            kh, kw = tap // 3, tap % 3
            np.fill_diagonal(wqdw_diag[ct, tap], wdw_g[ct * 128:(ct + 1) * 128, kh, kw])
    wdwm_diag = np.zeros((8, 9, 128, 128), f32)
    for g in range(8):
        for tap in range(9):
            kh, kw = tap // 3, tap % 3
            np.fill_diagonal(wdwm_diag[g, tap], w_mlp_dw[g * 128:(g + 1) * 128, 0, kh, kw])
    W1g = w_mlp1 * g_mlp[None, :]
    b1v = b_mlp1 + w_mlp1 @ b_mlp

    def lhsT_tiles(Wt, nk):  # W [out, in] -> [128, nk, out] partition-first lhsT
        st = np.stack([Wt.T[i * 128:(i + 1) * 128] for i in range(nk)])
        return np.ascontiguousarray(st.transpose(1, 0, 2)).astype(bf16)

    wqpw_h = lhsT_tiles(w_q_pw, 2)
    wproj_h = lhsT_tiles(w_proj, 2)
    wmlp1_h = lhsT_tiles(W1g, 2)
    wmlp2_h = lhsT_tiles(w_mlp2, 8)
    wqdw_h = np.ascontiguousarray(wqdw_diag.transpose(2, 0, 1, 3)).astype(bf16)
    wdwm_h = np.ascontiguousarray(wdwm_diag.transpose(2, 0, 1, 3)).astype(bf16)

    s2_core = []
    for core in range(NCORES):
        b, hf = core // 2, core % 2
        r0 = hf * R
        lo_ch = _chunk(low, b, r0, r0 + R, 2, 2)
        # kbd carries kq = k_h @ Wq_pw[32h:32h+32, :] (query 1x1 folded into K);
        # pvbd carries pv = v_h @ Wproj[:, 32h:32h+32]^T (proj folded into V).
        kbd = np.zeros((2, 128, 152), f32)
        pvbd = np.zeros((2, 128, 256), f32)
        onesbd = np.zeros((128, 4), f32)
        expd = np.zeros((4, 76), f32)
        bexp = np.zeros((128, 2), f32)
        for h in range(NH):
            hf2 = h // 4
            base = (h % 4) * NCL
            j = hf2 * 76 + base
            d0 = 32 * h
            kq = k[b, :, d0:d0 + 32] @ w_q_pw[d0:d0 + 32, :]        # [19, 256]
            pv = v[b, :, d0:d0 + 32] @ w_proj[:, d0:d0 + 32].T     # [19, 256]
            for n in range(NCL):
                kbd[0, :, j + n] = kq[n, 0:128]
                kbd[1, :, j + n] = kq[n, 128:256]
                pvbd[hf2, base + n, :] = pv[n, :]
            onesbd[base:base + NCL, h % 4] = 1.0
            expd[h % 4, base:base + NCL] = 1.0
            bexp[base:base + NCL, hf2] = -SCALE * lbh[b, h, :]

        zmask = np.ones((128, 2, TN), f32)
        if hf == 0:
            zmask[:, 0, 0:256] = 0.0      # tile 0: image rows -2, -1
        else:
            zmask[:, 1, 256:512] = 0.0    # tile 16: image rows 128, 129

        s2_core.append(dict(
            lo16=lo_ch.astype(bf16),
            ones=ones128.astype(bf16),
            kbd=np.ascontiguousarray(kbd.transpose(1, 0, 2)).astype(bf16),
            pvbd=np.ascontiguousarray(pvbd.transpose(1, 0, 2)).astype(bf16),
            onesbd=onesbd.astype(bf16),
            expd=expd.astype(f32), bexp=bexp.astype(f32),
            wqdw=wqdw_h,
            wmlp1=wmlp1_h, wdwm=wdwm_h, wmlp2=wmlp2_h,
            bprj=np.ascontiguousarray(b_proj.reshape(2, 128).T).astype(f32),
            b1=np.ascontiguousarray(b1v.reshape(8, 128).T).astype(f32),
            bdw=np.ascontiguousarray(b_mlp_dw.reshape(8, 128).T).astype(f32),
            b2=np.ascontiguousarray(b_mlp2.reshape(2, 128).T).astype(f32),
            zmask=zmask.astype(bf16),
        ))

    global _last_s2_inputs
    _last_s2_inputs = s2_core
    if _S2 is None:
        _S2 = build_stage2()
    res2 = run_bass_kernel_spmd(_S2, s2_core, core_ids=list(range(NCORES)))

    out = np.zeros((B, C, H, W), np.float32)
    for core in range(NCORES):
        b, hf = core // 2, core % 2
        o = res2.results[core]["OUT"]            # [128, 2, 8192]
        o = o.transpose(1, 0, 2).reshape(C, R, W)
        out[b, :, hf * R:(hf + 1) * R, :] = o
    return out

